# revision 23
# baseline (speedup 1.0000x reference)
"""DiT block kernel for 8x Trainium2 NeuronCores (Bass/Tile).

Sharding: row-parallel over the flattened (B,T)=4096 rows; 512 rows/core.
Cores 0-3 handle batch 0, cores 4-7 batch 1. MQA K/V is computed per-shard
and AllGather'd (in two row-chunks, launched as soon as the rows are
LayerNormed) within each 4-core batch group. Weights are replicated and
cast to bf16; LN stats and residual accumulation stay fp32, the modulated
residual h is carried in bf16.

Performance notes (vs the first working version):
  - modulation matmuls (M=1) are column-packed 4-wide via tile_position
  - LN affine chain runs in bf16 (DVE 2x modes), adds on DVE not GpSimd
    (GpSimd stays empty so the AllGather triggers fire immediately)
  - attention: scores psum tiles hold 2 key-tiles so exp runs at FD=1024;
    MM1 of head h+1 is interleaved with PV of head h instruction-by-
    instruction so the PE never sits behind a blocked queue head; softmax
    denominators use the ones-column trick + reciprocal_approx_fast
  - out-projection packs head pairs (K=128) and folds wo_b into the psum
    accumulation via a K=1 ones-row matmul (same for b2 in mlp2)
  - weight streams issue from sync/scalar/gpsimd queues, never stealing
    ScalarE time during attention (exp is the attention bottleneck)
"""

import os
import sys

sys.path.insert(0, "/opt/trn_rl_repo")

import numpy as np
import ml_dtypes

BF16 = ml_dtypes.bfloat16

B, T, F, H, D, M, C = 2, 2048, 1024, 16, 64, 4, 512
NCORES = 8
R = (B * T) // NCORES  # 512 rows per core
RB = R // 128  # 4 row blocks
FT = F // 128  # 8 feature tiles
MT = (H * D) // 128  # 8 head-pair tiles
MFT = (M * F) // 128  # 32 mlp hidden tiles
KT = T // 128  # 16 key tiles
EPS = 1e-5

_CACHE = {}

# HW-bisect feature flags (baseline-proven defaults)
V_RECIP = os.environ.get("V_RECIP", "exact")   # exact | fast (fast is broken on HW)
V_EXP = int(os.environ.get("V_EXP", "1024"))   # 512 | 1024
V_BIAS = os.environ.get("V_BIAS", "mm")        # dve | mm
V_OPACK = os.environ.get("V_OPACK", "dma")     # flat | dma
V_MLP8 = int(os.environ.get("V_MLP8", "1"))    # 1 = fp8 DoubleRow MLP, 0 = bf16


def _build_nc():
    import concourse.bass as bass
    import concourse.tile as tile
    from concourse import bacc, mybir
    from concourse.masks import make_identity
    from contextlib import ExitStack

    f32 = mybir.dt.float32
    f16 = mybir.dt.float16
    bf16 = mybir.dt.bfloat16
    fp8 = mybir.dt.float8e4
    DR = mybir.MatmulPerfMode.DoubleRow
    AF = mybir.ActivationFunctionType
    OP = mybir.AluOpType

    nc = bacc.Bacc(
        "TRN2",
        target_bir_lowering=False,
        debug=False,
        enable_asserts=False,
        num_devices=NCORES,
    )

    def dram(name, shape, dt, **kw):
        return nc.dram_tensor(name, shape, dt, **kw).ap()

    x_d = dram("x", [R, F], f32, kind="ExternalInput")
    cond_d = dram("cond", [C], bf16, kind="ExternalInput")
    wmod_d = dram("wmod", [C, 4 * F], bf16, kind="ExternalInput")
    modb_d = dram("modb", [4 * F], f32, kind="ExternalInput")
    lnv_d = dram("lnvec", [6, F], f32, kind="ExternalInput")
    wq_d = dram("wq", [MT, 128, FT * 128], bf16, kind="ExternalInput")
    wkv_d = dram("wkv", [F, 2 * D], bf16, kind="ExternalInput")
    wo_d = dram("wo", [H * D, F], bf16, kind="ExternalInput")
    wob_d = dram("wo_bias", [1, F], f32, kind="ExternalInput")
    if V_MLP8:
        w1_d = dram("w1", [MFT, 128, FT * 128], fp8, kind="ExternalInput")
        w2_d = dram("w2", [MFT // 2, 128, 2 * F], fp8, kind="ExternalInput")
    else:
        w1_d = dram("w1", [MFT, 128, FT * 128], bf16, kind="ExternalInput")
        w2_d = dram("w2", [M * F, F], bf16, kind="ExternalInput")
    b1_d = dram("b1", [M * F], f32, kind="ExternalInput")
    b2_d = dram("b2", [1, F], f32, kind="ExternalInput")
    y_d = dram("y", [R, F], f32, kind="ExternalOutput")

    groups = [[0, 1, 2, 3], [4, 5, 6, 7]]

    def bcast_row(ap_row, nparts=128):
        # [1, n] DRAM AP -> partition-broadcast [nparts, n]
        return bass.AP(
            tensor=ap_row.tensor,
            offset=ap_row.offset,
            ap=[[0, nparts]] + list(ap_row.ap[-1:]),
        )

    with tile.TileContext(nc) as tc, ExitStack() as ctx:
        consts = ctx.enter_context(tc.tile_pool(name="consts", bufs=1))
        work = ctx.enter_context(tc.tile_pool(name="work", bufs=2))
        persist = ctx.enter_context(tc.tile_pool(name="persist", bufs=1))
        wstr = ctx.enter_context(tc.tile_pool(name="wstr", bufs=3))
        dramp = ctx.enter_context(tc.tile_pool(name="dramp", bufs=1, space="DRAM"))
        # phase-scoped PSUM pools (8 banks each era)
        cm_psF = tc.tile_pool(name="psF", bufs=1, space="PSUM")
        psF = cm_psF.__enter__()

        # ---------------- constants ----------------
        ident = consts.tile([128, 128], bf16, name="ident")
        make_identity(nc, ident)
        ones16 = consts.tile([1, 128], f16, name="ones16")
        nc.vector.memset(ones16, 1.0)
        # fp32 ones row at partition 64 (softmax denom broadcast) and at
        # partition 0 (bias fold into psum accumulators)
        ones_dn = consts.tile([128, 64], f32, name="ones_dn")
        nc.vector.memset(ones_dn[64:65, :], 1.0)
        ones_dn16 = consts.tile([128, 64], f16, name="ones_dn16")
        nc.vector.memset(ones_dn16[64:65, :], 1.0)
        ones_b = consts.tile([1, 128], f32, name="ones_b")
        nc.vector.memset(ones_b, 1.0)
        epst = consts.tile([128, 1], f32, name="epst")
        nc.vector.memset(epst, EPS)

        cond_sb = consts.tile([128, 4], bf16, name="cond_sb")
        nc.sync.dma_start(out=cond_sb, in_=cond_d.rearrange("(a p) -> p a", p=128))
        b1_sb = consts.tile([128, MFT], f32, name="b1_sb")
        nc.scalar.dma_start(out=b1_sb, in_=b1_d.rearrange("(mt p) -> p mt", p=128))
        wkv_sb = consts.tile([128, FT, 2 * D], bf16, name="wkv_sb")
        nc.sync.dma_start(
            out=wkv_sb, in_=wkv_d.rearrange("(kt p) n -> p kt n", p=128)
        )

        anw_f = consts.tile([128, F], f32, name="anw_f")
        nc.scalar.dma_start(out=anw_f, in_=bcast_row(lnv_d[2:3, :]))
        anb_f = consts.tile([128, F], f32, name="anb_f")
        nc.scalar.dma_start(out=anb_f, in_=bcast_row(lnv_d[3:4, :]))
        anw_bc = consts.tile([128, F], bf16, name="anw_bc")
        nc.vector.tensor_copy(out=anw_bc, in_=anw_f)
        anb_bc = consts.tile([128, F], bf16, name="anb_bc")
        nc.vector.tensor_copy(out=anb_bc, in_=anb_f)
        # bias rows for the K=1 psum-fold matmuls
        wob_row = consts.tile([1, F], f32, name="wob_row")
        nc.scalar.dma_start(out=wob_row, in_=wob_d[0:1, :])
        b2_row = consts.tile([1, F], f32, name="b2_row")
        nc.scalar.dma_start(out=b2_row, in_=b2_d[0:1, :])
        if V_BIAS == "dve":
            wob_bc = consts.tile([128, F], f32, name="wob_bc")
            nc.scalar.dma_start(out=wob_bc, in_=bcast_row(wob_d[0:1, :]))
            b2_bc = consts.tile([128, F], f32, name="b2_bc")
            nc.scalar.dma_start(out=b2_bc, in_=bcast_row(b2_d[0:1, :]))

        # ---------------- phase 0: modulation vectors ----------------
        # modv = cond @ [gA | bA | gF | bF] + modb  -> [1, 4F] fp32, then
        # Wa = amod_nw*(1+gA), Ba = amod_nb*(1+gA)+bA (same for fmod),
        # PE-broadcast to [128, F] bf16 tiles. The four 512-col slices of
        # each half are column-packed onto distinct PE col-groups.
        cm_hera = tc.tile_pool(name="hera", bufs=1)
        hera = cm_hera.__enter__()
        cm_modtmp = tc.tile_pool(name="modtmp", bufs=1)
        modtmp = cm_modtmp.__enter__()

        lnr = {}
        for r in (0, 1, 4, 5):  # amod_nw/nb, fmod_nw/nb rows at partition 0
            lnr[r] = modtmp.tile([1, F], f32, name=f"lnr{r}")
            nc.scalar.dma_start(out=lnr[r], in_=lnv_d[r : r + 1, :])
        modb_sb = modtmp.tile([1, 4 * F], f32, name="modb_sb")
        nc.scalar.dma_start(out=modb_sb, in_=modb_d.rearrange("(a f) -> a f", a=1))
        modv = modtmp.tile([1, 4 * F], f32, name="modv")
        for grp in range(2):
            wm_tiles = []
            for chd in range(4):
                wm = modtmp.tile(
                    [128, 2048], bf16, tag="wm", bufs=2, name=f"wm{grp}_{chd}"
                )
                nc.sync.dma_start(
                    out=wm,
                    in_=wmod_d[
                        chd * 128 : (chd + 1) * 128, grp * 2048 : (grp + 1) * 2048
                    ],
                )
                wm_tiles.append(wm)
            pms = [
                psF.tile([128, 512], f32, tag="pmod", bufs=4, name=f"pm{grp}_{j}")
                for j in range(4)
            ]
            for chd in range(4):
                for j in range(4):
                    nc.tensor.matmul(
                        pms[j][0:1, :],
                        cond_sb[:, chd : chd + 1],
                        wm_tiles[chd][:, j * 512 : (j + 1) * 512],
                        start=(chd == 0),
                        stop=(chd == 3),
                    )
            for j in range(4):
                nb = grp * 4 + j
                nc.vector.tensor_add(
                    out=modv[:, nb * 512 : (nb + 1) * 512],
                    in0=pms[j][0:1, :],
                    in1=modb_sb[:, nb * 512 : (nb + 1) * 512],
                )

        tmpv = modtmp.tile([1, F], f32, name="tmpv")
        bc = {}
        modv16 = modtmp.tile([1, 4 * F], f16, name="modv16")

        def finalize_mod(g_off, b_off, nw_row, nb_row, w_name, b_name):
            g_sl = modv[:, g_off : g_off + F]
            b_sl = modv[:, b_off : b_off + F]
            nc.scalar.add(out=g_sl, in_=g_sl, add=1.0)
            nc.vector.tensor_mul(out=tmpv, in0=g_sl, in1=lnr[nb_row])
            with nc.allow_low_precision(reason="f16 staging for PE broadcast"):
                nc.vector.tensor_add(
                    out=modv16[:, b_off : b_off + F], in0=tmpv, in1=b_sl
                )
                nc.vector.tensor_mul(
                    out=modv16[:, g_off : g_off + F], in0=g_sl, in1=lnr[nw_row]
                )
            for off, nm in ((g_off, w_name), (b_off, b_name)):
                bt = consts.tile([128, F], bf16, name=nm)
                for hf in range(2):
                    pb = psF.tile([128, 512], f32, tag="pmod", bufs=4, name="pbc")
                    nc.tensor.matmul(
                        pb,
                        ones16,
                        modv16[:, off + hf * 512 : off + (hf + 1) * 512],
                        start=True,
                        stop=True,
                    )
                    nc.scalar.activation(
                        bt[:, hf * 512 : (hf + 1) * 512], pb, AF.Copy
                    )
                bc[nm] = bt

        finalize_mod(0, F, 0, 1, "Wa_bc", "Ba_bc")

        # ---------------- phase 1: adaLN-1 + attn-LN + kv + gather ----------------

        hT = [
            persist.tile([128, R], bf16, tag=f"hT{ft}", name=f"hT{ft}")
            for ft in range(FT)
        ]
        h_res = [hera.tile([128, F], bf16, name=f"h{rb}") for rb in range(RB)]
        kvT_sb = hera.tile([128, R], bf16, name="kvT_sb")

        kv_bounce = [dramp.tile([2 * D, 256], bf16, name=f"kvb{c}") for c in range(2)]
        kv_all = [dramp.tile([4 * 2 * D, 256], bf16, name=f"kva{c}") for c in range(2)]

        def layer_norm_stats(src):
            stats = work.tile([128, 2, 6], f32, tag="stats", name="stats")
            for sg in range(2):
                nc.vector.bn_stats(
                    out=stats[:, sg, :], in_=src[:, sg * 512 : (sg + 1) * 512]
                )
            mv = work.tile([128, 2], f32, tag="mv", name="mv")
            nc.vector.bn_aggr(out=mv, in_=stats)
            rstd = work.tile([128, 1], f32, tag="rstd", name="rstd")
            nc.scalar.activation(
                out=rstd, in_=mv[:, 1:2], func=AF.Sqrt, bias=epst, scale=1.0
            )
            nc.vector.reciprocal(out=rstd, in_=rstd)
            return mv, rstd

        def transpose_to_h2(hsrc_bf, rb, tpool):
            for ft in range(FT):
                pt = tpool.tile([128, 128], bf16, tag="pt", bufs=2, name="ptt2")
                nc.tensor.transpose(
                    pt, hsrc_bf[:, ft * 128 : (ft + 1) * 128], ident
                )
                nc.scalar.activation(
                    out=h2all[:, ft, rb * 128 : (rb + 1) * 128],
                    in_=pt,
                    func=AF.Copy,
                )

        def transpose_to(hsrc_bf, hT_tiles, rb, tpool):
            for ft in range(FT):
                pt = tpool.tile([128, 128], bf16, tag="pt", bufs=2, name="ptt")
                nc.tensor.transpose(
                    pt, hsrc_bf[:, ft * 128 : (ft + 1) * 128], ident
                )
                nc.scalar.activation(
                    out=hT_tiles[ft][:, rb * 128 : (rb + 1) * 128],
                    in_=pt,
                    func=AF.Copy,
                )

        with nc.named_scope("p1_ln"):
            xns = []
            for rb in range(RB):
                x_rb = work.tile([128, F], f32, tag="x", bufs=4, name="x_rb")
                nc.sync.dma_start(out=x_rb, in_=x_d[rb * 128 : (rb + 1) * 128, :])
                mv, rstd = layer_norm_stats(x_rb)
                xn = hera.tile([128, F], bf16, name=f"xnp{rb}")
                nc.vector.tensor_scalar(
                    out=xn,
                    in0=x_rb,
                    scalar1=mv[:, 0:1],
                    scalar2=rstd,
                    op0=OP.subtract,
                    op1=OP.mult,
                )
                xns.append(xn)
            for rb in range(RB):
                h0 = work.tile([128, F], bf16, tag="h0", name="h0")
                nc.vector.tensor_mul(out=h0, in0=xns[rb], in1=bc["Wa_bc"])
                nc.vector.tensor_add(out=h_res[rb], in0=h0, in1=bc["Ba_bc"])
                # attn-LN
                mv2, rstd2 = layer_norm_stats(h_res[rb])
                xn2 = work.tile([128, F], bf16, tag="xn", name="xn2")
                nc.vector.tensor_scalar(
                    out=xn2,
                    in0=h_res[rb],
                    scalar1=mv2[:, 0:1],
                    scalar2=rstd2,
                    op0=OP.subtract,
                    op1=OP.mult,
                )
                hn1 = work.tile([128, F], bf16, tag="h0", name="hn1")
                nc.vector.tensor_mul(out=hn1, in0=xn2, in1=anw_bc)
                hn_bf = work.tile([128, F], bf16, tag="hnbf", name="hn_bf")
                nc.vector.tensor_add(out=hn_bf, in0=hn1, in1=anb_bc)
                transpose_to(hn_bf, hT, rb, psF)
                # kv projection for this row block
                pkv = psF.tile([128, 512], f32, tag="pkq", bufs=2, name="pkv")
                for kt in range(FT):
                    nc.tensor.matmul(
                        pkv[:, 0:128],
                        wkv_sb[:, kt, :],
                        hT[kt][:, rb * 128 : (rb + 1) * 128],
                        start=(kt == 0),
                        stop=(kt == FT - 1),
                    )
                nc.scalar.activation(
                    out=kvT_sb[:, rb * 128 : (rb + 1) * 128],
                    in_=pkv[:, 0:128],
                    func=AF.Copy,
                )
                if rb % 2 == 1:
                    c = rb // 2
                    with nc.named_scope(f"gather{c}"):
                        nc.sync.dma_start(
                            out=kv_bounce[c], in_=kvT_sb[:, c * 256 : (c + 1) * 256]
                        )
                        nc.gpsimd.collective_compute(
                            "AllGather",
                            OP.bypass,
                            replica_groups=groups,
                            ins=[kv_bounce[c][:, :]],
                            outs=[kv_all[c][:, :]],
                        )

        finalize_mod(2 * F, 3 * F, 4, 5, "Wf_bc", "Bf_bc")
        cm_modtmp.__exit__(None, None, None)
        cm_aera = tc.tile_pool(name="aera", bufs=1)
        aera = cm_aera.__enter__()

        # ---------------- phase 2: q proj ----------------
        qT = [aera.tile([128, R], bf16, name=f"qT{mt}") for mt in range(MT)]
        with nc.named_scope("qproj"):
            for mt in range(MT):
                wqblk = wstr.tile(
                    [128, FT * 128], bf16, tag="wqb", bufs=2, name="wqblk"
                )
                nc.sync.dma_start(out=wqblk, in_=wq_d[mt])
                pq = psF.tile([128, 512], f32, tag="pkq", bufs=2, name="pq")
                for kt in range(FT):
                    nc.tensor.matmul(
                        pq,
                        wqblk[:, kt * 128 : (kt + 1) * 128],
                        hT[kt],
                        start=(kt == 0),
                        stop=(kt == FT - 1),
                    )
                # fold the attention 1/sqrt(D)=0.125 scale into q
                nc.scalar.activation(out=qT[mt], in_=pq, func=AF.Copy, scale=0.125)

        # ---------------- phase 3: kT / v_ext assembly ----------------
        # k^T duplicated into both partition halves so MM1's lhsT can share
        # the rhs (q head slice) base partition for even and odd heads.
        kT = aera.tile([128, T], bf16, name="kT")
        # fp8 DoubleRow PV weights: vd[j] holds 16*v for key tiles (2j, 2j+1)
        # in [Ki, ko, col] layout, col 64 = 16.0 (the softmax-denominator ones
        # column, pre-scaled to keep v in fp8 normal range).
        vd = [aera.tile([128, 2, 80], fp8, name=f"vd{j}") for j in range(KT // 2)]
        with nc.named_scope("asm"):
            for j in range(KT // 2):
                nc.vector.memset(vd[j][:, :, 64:65], 16.0)
            for c in range(2):
                for r in range(4):
                    for hp in (0, 64):
                        nc.sync.dma_start(
                            out=kT[
                                hp : hp + 64, r * 512 + c * 256 : r * 512 + (c + 1) * 256
                            ],
                            in_=kv_all[c][r * 128 : r * 128 + 64, :],
                        )
                    vT_sb = work.tile([64, 256], bf16, tag="vTs", name="vT_sb")
                    nc.sync.dma_start(
                        out=vT_sb, in_=kv_all[c][r * 128 + 64 : (r + 1) * 128, :]
                    )
                    for cc in range(2):
                        kt = r * 4 + c * 2 + cc
                        ptv = psF.tile(
                            [128, 128], bf16, tag="pt", bufs=2, name="ptv"
                        )
                        nc.tensor.matmul(
                            ptv[:, 0:64],
                            vT_sb[:, cc * 128 : (cc + 1) * 128],
                            ident[0:64, 0:64],
                            is_transpose=True,
                        )
                        nc.scalar.activation(
                            out=vd[kt // 2][:, kt % 2, 0:64],
                            in_=ptv[:, 0:64],
                            func=AF.Copy,
                            scale=16.0,
                        )

        # prefetch wo during attention (sync queue)
        if V_OPACK == "dma":
            woc = [aera.tile([128, F], bf16, name=f"woc{m}") for m in range(MT)]
            for m in range(MT):
                nc.sync.dma_start(out=woc[m], in_=wo_d[m * 128 : (m + 1) * 128, :])
        else:
            woc_f = [aera.tile([64, F], bf16, name=f"wocf{k}") for k in range(H)]
            for k in range(H):
                nc.sync.dma_start(
                    out=woc_f[k], in_=wo_d[k * 64 : (k + 1) * 64, :]
                )

        cm_psF.__exit__(None, None, None)
        cm_psAT = tc.tile_pool(name="psAT", bufs=1, space="PSUM")
        psAT = cm_psAT.__enter__()

        # ---------------- phase 4: attention ----------------
        # Per head: MM1 into [128,1024] psum tiles (2 key tiles per bank
        # pair), exp at FD=1024 psum->sbuf bf16, PV with ones-column denom.
        # PE instruction order interleaves MM1(h) with PV(h-1) so the queue
        # head is never blocked on exp.
        if V_OPACK == "dma":
            outTp = [aera.tile([128, R], bf16, name=f"outTp{m}") for m in range(MT)]
        else:
            outT_f = [aera.tile([64, R], bf16, name=f"outTf{k}") for k in range(H)]
        probs_all = {}
        probs_sc = {}

        def emit_mm1(hi, j):
            # scores for key tiles (2j, 2j+1) of head hi; two 1-bank psum
            # tiles exp'd separately into halves of the fp8 probs pair tile
            mt, hp = hi // 2, (hi % 2) * 64
            pr = aera.tile(
                [128, 2, 512], fp8, tag="probs", bufs=24, name=f"pr{hi}_{j}"
            )
            for cc in range(2):
                kt = 2 * j + cc
                sc = psAT.tile(
                    [128, 512], f32, tag="sc", bufs=4, name=f"sc{hi}_{j}_{cc}"
                )
                nc.tensor.matmul(
                    sc,
                    kT[hp : hp + 64, kt * 128 : (kt + 1) * 128],
                    qT[mt][hp : hp + 64, :],
                    start=True,
                    stop=True,
                )
                nc.scalar.activation(out=pr[:, cc, :], in_=sc, func=AF.Exp)
            probs_all[(hi, j)] = pr

        def emit_pv(hi, j, po):
            # fp8 DoubleRow: contracts key tiles 2j and 2j+1 in one matmul
            pr = probs_all.pop((hi, j))
            nc.tensor.matmul(
                po[0:65, :],
                vd[j][:, :, 0:65],
                pr,
                start=(j == 0),
                stop=(j == 7),
                perf_mode=DR,
            )

        def emit_normalize(hi, po):
            # reciprocal of the denominator row, broadcast to 64 partitions
            # via a DRAM bounce (no PE involvement -> no PE queue blocking)
            m, hp = hi // 2, (hi % 2) * 64
            rcp_row = work.tile([128, R], f16, tag="rcp", name="rcp_row")
            with nc.allow_low_precision(reason="f16 softmax denom"):
                nc.vector.reciprocal(out=rcp_row[64:65, :], in_=po[64:65, :])
            rden = dramp.tile([1, R], f16, tag="rden", bufs=2, name=f"rden{hi}")
            nc.sync.dma_start(out=rden, in_=rcp_row[64:65, :])
            rcpb = work.tile([64, R], f16, tag="rcpb", name="rcpb")
            nc.sync.dma_start(out=rcpb, in_=bcast_row(rden[0:1, :], nparts=64))
            if hp == 0:
                nc.vector.tensor_mul(
                    out=outTp[m][0:64, :], in0=po[0:64, :], in1=rcpb
                )
            else:
                # DVE cannot shift partitions; stage at base 0 then DMA up
                oT = work.tile([64, R], bf16, tag="oT", name="oT")
                nc.vector.tensor_mul(out=oT, in0=po[0:64, :], in1=rcpb)
                nc.sync.dma_start(out=outTp[m][64:128, :], in_=oT)

        JSEQ = [0, 2, 4, 6, 1, 3, 5, 7]  # chunk-0 key tiles first
        with nc.named_scope("attn"):
            po_t = {}
            for j in JSEQ:
                emit_mm1(0, j)
                emit_mm1(1, j)
            for m in range(MT):
                h0, h1 = 2 * m, 2 * m + 1
                if m > 0:
                    emit_normalize(h0 - 2, po_t.pop(h0 - 2))
                    emit_normalize(h1 - 2, po_t.pop(h1 - 2))
                for hi in (h0, h1):
                    po_t[hi] = psAT.tile(
                        [128, 512], f32, tag="po", bufs=4, name=f"po{hi}"
                    )
                for j in JSEQ:
                    emit_pv(h0, j, po_t[h0])
                    emit_pv(h1, j, po_t[h1])
                    if m + 1 < MT:
                        emit_mm1(h0 + 2, j)
                        emit_mm1(h1 + 2, j)
            emit_normalize(H - 2, po_t.pop(H - 2))
            emit_normalize(H - 1, po_t.pop(H - 1))

        cm_psAT.__exit__(None, None, None)
        cm_psO = tc.tile_pool(name="psO", bufs=1, space="PSUM")
        psO = cm_psO.__enter__()

        # ---------------- phase 5: out proj (head pairs) + residual -> x1 ----------------
        x1 = [persist.tile([128, F], f32, name=f"x1_{rt}") for rt in range(RB)]
        with nc.named_scope("oproj"):
            px1 = {}
            for rt in range(RB):
                px1[rt] = psO.tile([128, F], f32, tag="px1", bufs=4, name=f"px1_{rt}")
            nchunk = MT if V_OPACK == "dma" else H
            for m in range(nchunk):
                for rt in range(RB):
                    for nh in range(2):
                        if V_OPACK == "dma":
                            lhsT = outTp[m][:, rt * 128 : (rt + 1) * 128]
                            rhs = woc[m][:, nh * 512 : (nh + 1) * 512]
                        else:
                            lhsT = outT_f[m][:, rt * 128 : (rt + 1) * 128]
                            rhs = woc_f[m][:, nh * 512 : (nh + 1) * 512]
                        nc.tensor.matmul(
                            px1[rt][:, nh * 512 : (nh + 1) * 512],
                            lhsT,
                            rhs,
                            start=(m == 0),
                            stop=(V_BIAS == "dve" and m == nchunk - 1),
                        )
            for rt in range(RB):
                if V_BIAS == "mm":
                    for nh in range(2):
                        # fold wo_b into the accumulator: += ones^T(rows) x wob
                        nc.tensor.matmul(
                            px1[rt][:, nh * 512 : (nh + 1) * 512],
                            ones_b,
                            wob_row[:, nh * 512 : (nh + 1) * 512],
                            start=False,
                            stop=True,
                        )
                nc.vector.tensor_add(out=x1[rt], in0=px1[rt], in1=h_res[rt])
                if V_BIAS == "dve":
                    nc.vector.tensor_add(out=x1[rt], in0=x1[rt], in1=wob_bc)

        cm_aera.__exit__(None, None, None)
        cm_hera.__exit__(None, None, None)

        cm_psO.__exit__(None, None, None)
        cm_psM = tc.tile_pool(name="psM", bufs=1, space="PSUM")
        psM = cm_psM.__enter__()

        # ---------------- phase 6: adaLN-2 + transpose ----------------
        h2all = persist.tile([128, FT, R], fp8 if V_MLP8 else bf16, name="h2all")
        with nc.named_scope("aln2"):
            for rt in range(RB):
                mv, rstd = layer_norm_stats(x1[rt])
                xn = work.tile([128, F], bf16, tag="xn", name="xn3")
                nc.vector.tensor_scalar(
                    out=xn,
                    in0=x1[rt],
                    scalar1=mv[:, 0:1],
                    scalar2=rstd,
                    op0=OP.subtract,
                    op1=OP.mult,
                )
                h21 = work.tile([128, F], bf16, tag="h0", name="h21")
                nc.vector.tensor_mul(out=h21, in0=xn, in1=bc["Wf_bc"])
                h2_bf = work.tile([128, F], bf16, tag="hnbf", name="h2_bf")
                nc.vector.tensor_add(out=h2_bf, in0=h21, in1=bc["Bf_bc"])
                transpose_to_h2(h2_bf, rt, psM)

        # ---------------- phase 7+8: mlp1 + gelu, mlp2 interleaved ----------------
        cm_mlp = tc.tile_pool(name="mlpera", bufs=1)
        mlpera = cm_mlp.__enter__()
        g1all = mlpera.tile([128, MFT, R], fp8 if V_MLP8 else bf16, name="g1all")
        pf = {}
        with nc.named_scope("mlp"):
            if V_MLP8:
                w2f = [
                    mlpera.tile([128, 2, F], fp8, name=f"w2f{k}")
                    for k in range(MFT // 2)
                ]
                for k in range(MFT // 2):
                    eng = nc.gpsimd if k % 2 == 0 else nc.scalar
                    eng.dma_start(out=w2f[k], in_=w2_d[k])
            for rt in range(RB):
                pf[rt] = psM.tile([128, 512], f32, tag="pf", bufs=4, name=f"pf{rt}")

            def mlp2_chunk8(k, fh, pfd):
                # fp8 DR mlp2 accumulation for hidden pair k, F-half fh
                for rt in range(RB):
                    nc.tensor.matmul(
                        pfd[rt],
                        g1all[:, 2 * k : 2 * k + 2, rt * 128 : (rt + 1) * 128],
                        w2f[k][:, :, fh * 512 : (fh + 1) * 512],
                        start=(k == 0),
                        stop=False,
                        perf_mode=DR,
                    )

            def mlp2_chunk16(mt, fh, pfd):
                # bf16 mlp2 accumulation for hidden tile mt
                w2c = wstr.tile([128, 512], bf16, tag="w2c", bufs=3, name="w2c")
                eng = nc.gpsimd if mt % 2 == 0 else nc.scalar
                eng.dma_start(
                    out=w2c,
                    in_=w2_d[mt * 128 : (mt + 1) * 128, fh * 512 : (fh + 1) * 512],
                )
                for rt in range(RB):
                    nc.tensor.matmul(
                        pfd[rt],
                        g1all[:, mt, rt * 128 : (rt + 1) * 128],
                        w2c,
                        start=(mt == 0),
                        stop=False,
                    )

            for mt in range(MFT):
                if V_MLP8:
                    w1blk = wstr.tile(
                        [128, FT // 2, 2, 128], fp8, tag="w1b", bufs=3, name="w1blk"
                    )
                    eng = nc.sync if mt % 2 == 0 else nc.scalar
                    eng.dma_start(
                        out=w1blk,
                        in_=w1_d[mt].rearrange(
                            "p (j two m) -> p j two m", j=FT // 2, two=2
                        ),
                    )
                else:
                    w1blk = wstr.tile(
                        [128, FT * 128], bf16, tag="w1b", bufs=3, name="w1blk"
                    )
                    eng = nc.sync if mt % 2 == 0 else nc.scalar
                    eng.dma_start(out=w1blk, in_=w1_d[mt])
                pg = psM.tile([128, 512], f32, tag="pg", bufs=2, name="pg")
                if V_MLP8:
                    for j in range(FT // 2):
                        nc.tensor.matmul(
                            pg,
                            w1blk[:, j],
                            h2all[:, 2 * j : 2 * j + 2, :],
                            start=(j == 0),
                            stop=(j == FT // 2 - 1),
                            perf_mode=DR,
                        )
                else:
                    for kt in range(FT):
                        nc.tensor.matmul(
                            pg,
                            w1blk[:, kt * 128 : (kt + 1) * 128],
                            h2all[:, kt, :],
                            start=(kt == 0),
                            stop=(kt == FT - 1),
                        )
                nc.scalar.activation(
                    out=g1all[:, mt, :],
                    in_=pg,
                    func=AF.Gelu,
                    bias=b1_sb[:, mt : mt + 1],
                    scale=(1.0 / 256.0) if V_MLP8 else 1.0,
                )
                if V_MLP8:
                    if mt % 2 == 1 and mt > 1:
                        mlp2_chunk8((mt - 2) // 2, 0, pf)
                elif mt > 0:
                    mlp2_chunk16(mt - 1, 0, pf)
            if V_MLP8:
                mlp2_chunk8(MFT // 2 - 1, 0, pf)
            else:
                mlp2_chunk16(MFT - 1, 0, pf)
            oscale = (1.0 / 256.0) if V_MLP8 else 1.0
            for rt in range(RB):
                nc.tensor.matmul(
                    pf[rt], ones_b, b2_row[:, 0:512], start=False, stop=True
                )
                yh1 = work.tile([128, 512], f32, tag="yh1", name="yh1")
                nc.scalar.activation(out=yh1, in_=pf[rt], func=AF.Copy, scale=oscale)
                yh = work.tile([128, 512], f32, tag="yh", name="yh")
                nc.vector.tensor_add(out=yh, in0=yh1, in1=x1[rt][:, 0:512])
                nc.sync.dma_start(out=y_d[rt * 128 : (rt + 1) * 128, 0:512], in_=yh)
            # second half of mlp2
            pf2 = {}
            for rt in range(RB):
                pf2[rt] = psM.tile([128, 512], f32, tag="pf", bufs=4, name=f"pf2{rt}")
            if V_MLP8:
                for k in range(MFT // 2):
                    mlp2_chunk8(k, 1, pf2)
            else:
                for mt in range(MFT):
                    mlp2_chunk16(mt, 1, pf2)
            for rt in range(RB):
                nc.tensor.matmul(
                    pf2[rt], ones_b, b2_row[:, 512:1024], start=False, stop=True
                )
                yh1 = work.tile([128, 512], f32, tag="yh1", name="yh1b")
                nc.scalar.activation(out=yh1, in_=pf2[rt], func=AF.Copy, scale=oscale)
                yh = work.tile([128, 512], f32, tag="yh", name="yh2")
                nc.vector.tensor_add(out=yh, in0=yh1, in1=x1[rt][:, 512:1024])
                nc.sync.dma_start(
                    out=y_d[rt * 128 : (rt + 1) * 128, 512:1024], in_=yh
                )
        cm_mlp.__exit__(None, None, None)
        cm_psM.__exit__(None, None, None)

    nc.compile()
    return nc


def _prep_in_maps(inputs):
    f32 = np.float32
    wmod = np.concatenate(
        [inputs["amod_gw"], inputs["amod_bw"], inputs["fmod_gw"], inputs["fmod_bw"]],
        axis=1,
    ).astype(BF16)
    modb = np.concatenate(
        [inputs["amod_gb"], inputs["amod_bb"], inputs["fmod_gb"], inputs["fmod_bb"]]
    ).astype(f32)
    lnvec = np.stack(
        [
            inputs["amod_nw"],
            inputs["amod_nb"],
            inputs["attn_nw"],
            inputs["attn_nb"],
            inputs["fmod_nw"],
            inputs["fmod_nb"],
        ]
    ).astype(f32)
    wq_t = np.ascontiguousarray(
        np.asarray(inputs["wq"]).astype(BF16).reshape(FT, 128, MT, 128)
        .transpose(2, 1, 0, 3).reshape(MT, 128, FT * 128)
    )
    FP8 = ml_dtypes.float8_e4m3
    if int(os.environ.get("V_MLP8", "1")):
        # w1 * 256 in DoubleRow layout [mt, i, (j two m)]
        w1_t = np.ascontiguousarray(
            np.clip(np.asarray(inputs["w1"], np.float32) * 256.0, -240, 240)
            .reshape(FT // 2, 2, 128, MFT, 128)
            .transpose(3, 2, 0, 1, 4).reshape(MFT, 128, FT * 128)
        ).astype(FP8)
        # w2 * 256 in DoubleRow rhs layout [k, i, (two f)]
        w2_t = np.ascontiguousarray(
            np.clip(np.asarray(inputs["w2"], np.float32) * 256.0, -240, 240)
            .reshape(MFT // 2, 2, 128, F)
            .transpose(0, 2, 1, 3).reshape(MFT // 2, 128, 2 * F)
        ).astype(FP8)
        b2_t = np.asarray(inputs["b2"]).astype(f32).reshape(1, F) * 256.0
    else:
        w1_t = np.ascontiguousarray(
            np.asarray(inputs["w1"]).astype(BF16).reshape(FT, 128, MFT, 128)
            .transpose(2, 1, 0, 3).reshape(MFT, 128, FT * 128)
        )
        w2_t = np.asarray(inputs["w2"]).astype(BF16)
        b2_t = np.asarray(inputs["b2"]).astype(f32).reshape(1, F)
    shared = dict(
        wmod=wmod,
        modb=modb,
        lnvec=lnvec,
        wq=wq_t,
        wkv=np.asarray(inputs["wkv"]).astype(BF16),
        wo=np.asarray(inputs["wo"]).astype(BF16),
        wo_bias=np.asarray(inputs["wo_b"]).astype(f32).reshape(1, F),
        w1=w1_t,
        b1=np.asarray(inputs["b1"]).astype(f32),
        w2=w2_t,
        b2=b2_t,
    )
    x = np.asarray(inputs["x"]).astype(f32)
    cond = np.asarray(inputs["cond"]).astype(BF16)
    in_maps = []
    for c in range(NCORES):
        b, r0 = c // 4, (c % 4) * R
        m = dict(shared)
        m["x"] = np.ascontiguousarray(x[b, r0 : r0 + R, :])
        m["cond"] = np.ascontiguousarray(cond[b])
        in_maps.append(m)
    return in_maps


def _run(inputs, trace=False, tmpdir=None):
    from concourse.bass_utils import run_bass_kernel_spmd

    if "nc" not in _CACHE:
        _CACHE["nc"] = _build_nc()
    nc = _CACHE["nc"]
    in_maps = _prep_in_maps(inputs)
    res = run_bass_kernel_spmd(
        nc, in_maps, core_ids=list(range(NCORES)), trace=trace, tmpdir=tmpdir
    )
    y = np.empty((B, T, F), np.float32)
    for c in range(NCORES):
        b, r0 = c // 4, (c % 4) * R
        y[b, r0 : r0 + R, :] = res.results[c]["y"]
    return y, res


def kernel(**inputs) -> np.ndarray:
    y, _ = _run(inputs, trace=False)
    return y


if __name__ == "__main__":
    _build_nc()
    print("build OK")


# revision 27
# speedup vs baseline: 1.0235x; 1.0235x over previous
"""DiT block kernel for 8x Trainium2 NeuronCores (Bass/Tile).

Sharding: row-parallel over the flattened (B,T)=4096 rows; 512 rows/core.
Cores 0-3 handle batch 0, cores 4-7 batch 1. MQA K/V is computed per-shard
and AllGather'd (in two row-chunks, launched as soon as the rows are
LayerNormed) within each 4-core batch group. Weights are replicated and
cast to bf16; LN stats and residual accumulation stay fp32, the modulated
residual h is carried in bf16.

Performance notes (vs the first working version):
  - modulation matmuls (M=1) are column-packed 4-wide via tile_position
  - LN affine chain runs in bf16 (DVE 2x modes), adds on DVE not GpSimd
    (GpSimd stays empty so the AllGather triggers fire immediately)
  - attention: scores psum tiles hold 2 key-tiles so exp runs at FD=1024;
    MM1 of head h+1 is interleaved with PV of head h instruction-by-
    instruction so the PE never sits behind a blocked queue head; softmax
    denominators use the ones-column trick + reciprocal_approx_fast
  - out-projection packs head pairs (K=128) and folds wo_b into the psum
    accumulation via a K=1 ones-row matmul (same for b2 in mlp2)
  - weight streams issue from sync/scalar/gpsimd queues, never stealing
    ScalarE time during attention (exp is the attention bottleneck)
"""

import os
import sys

sys.path.insert(0, "/opt/trn_rl_repo")

import numpy as np
import ml_dtypes

BF16 = ml_dtypes.bfloat16

B, T, F, H, D, M, C = 2, 2048, 1024, 16, 64, 4, 512
NCORES = 8
R = (B * T) // NCORES  # 512 rows per core
RB = R // 128  # 4 row blocks
FT = F // 128  # 8 feature tiles
MT = (H * D) // 128  # 8 head-pair tiles
MFT = (M * F) // 128  # 32 mlp hidden tiles
KT = T // 128  # 16 key tiles
EPS = 1e-5

_CACHE = {}

# HW-bisect feature flags (baseline-proven defaults)
V_RECIP = os.environ.get("V_RECIP", "exact")   # exact | fast (fast is broken on HW)
V_EXP = int(os.environ.get("V_EXP", "1024"))   # 512 | 1024
V_BIAS = os.environ.get("V_BIAS", "mm")        # dve | mm
V_OPACK = os.environ.get("V_OPACK", "dma")     # flat | dma
V_MLP8 = int(os.environ.get("V_MLP8", "0"))    # 1 = fp8 DoubleRow MLP, 0 = bf16


def _build_nc():
    import concourse.bass as bass
    import concourse.tile as tile
    from concourse import bacc, mybir
    from concourse.masks import make_identity
    from contextlib import ExitStack

    f32 = mybir.dt.float32
    f16 = mybir.dt.float16
    bf16 = mybir.dt.bfloat16
    fp8 = mybir.dt.float8e4
    DR = mybir.MatmulPerfMode.DoubleRow
    AF = mybir.ActivationFunctionType
    OP = mybir.AluOpType

    nc = bacc.Bacc(
        "TRN2",
        target_bir_lowering=False,
        debug=False,
        enable_asserts=False,
        num_devices=NCORES,
    )

    def dram(name, shape, dt, **kw):
        return nc.dram_tensor(name, shape, dt, **kw).ap()

    x_d = dram("x", [R, F], f32, kind="ExternalInput")
    cond_d = dram("cond", [C], bf16, kind="ExternalInput")
    wmod_d = dram("wmod", [C, 4 * F], bf16, kind="ExternalInput")
    modb_d = dram("modb", [4 * F], f32, kind="ExternalInput")
    lnv_d = dram("lnvec", [6, F], f32, kind="ExternalInput")
    wq_d = dram("wq", [MT, 128, FT * 128], bf16, kind="ExternalInput")
    wkv_d = dram("wkv", [F, 2 * D], bf16, kind="ExternalInput")
    wo_d = dram("wo", [H * D, F], bf16, kind="ExternalInput")
    wob_d = dram("wo_bias", [1, F], f32, kind="ExternalInput")
    if V_MLP8:
        w1_d = dram("w1", [MFT, 128, FT * 128], fp8, kind="ExternalInput")
        w2_d = dram("w2", [MFT // 2, 128, 2 * F], fp8, kind="ExternalInput")
    else:
        w1_d = dram("w1", [MFT, 128, FT * 128], bf16, kind="ExternalInput")
        w2_d = dram("w2", [M * F, F], bf16, kind="ExternalInput")
    b1_d = dram("b1", [M * F], f32, kind="ExternalInput")
    b2_d = dram("b2", [1, F], f32, kind="ExternalInput")
    y_d = dram("y", [R, F], f32, kind="ExternalOutput")

    groups = [[0, 1, 2, 3], [4, 5, 6, 7]]

    def bcast_row(ap_row, nparts=128):
        # [1, n] DRAM AP -> partition-broadcast [nparts, n]
        return bass.AP(
            tensor=ap_row.tensor,
            offset=ap_row.offset,
            ap=[[0, nparts]] + list(ap_row.ap[-1:]),
        )

    with tile.TileContext(nc) as tc, ExitStack() as ctx:
        consts = ctx.enter_context(tc.tile_pool(name="consts", bufs=1))
        work = ctx.enter_context(tc.tile_pool(name="work", bufs=2))
        persist = ctx.enter_context(tc.tile_pool(name="persist", bufs=1))
        wstr = ctx.enter_context(tc.tile_pool(name="wstr", bufs=3))
        dramp = ctx.enter_context(tc.tile_pool(name="dramp", bufs=1, space="DRAM"))
        # phase-scoped PSUM pools (8 banks each era)
        cm_psF = tc.tile_pool(name="psF", bufs=1, space="PSUM")
        psF = cm_psF.__enter__()

        # ---------------- constants ----------------
        ident = consts.tile([128, 128], bf16, name="ident")
        make_identity(nc, ident)
        ones16 = consts.tile([1, 128], f16, name="ones16")
        nc.vector.memset(ones16, 1.0)
        # fp32 ones row at partition 64 (softmax denom broadcast) and at
        # partition 0 (bias fold into psum accumulators)
        ones_dn = consts.tile([128, 64], f32, name="ones_dn")
        nc.vector.memset(ones_dn[64:65, :], 1.0)
        ones_dn16 = consts.tile([128, 64], f16, name="ones_dn16")
        nc.vector.memset(ones_dn16[64:65, :], 1.0)
        ones_b = consts.tile([1, 128], f32, name="ones_b")
        nc.vector.memset(ones_b, 1.0)
        epst = consts.tile([128, 1], f32, name="epst")
        nc.vector.memset(epst, EPS)

        cond_sb = consts.tile([128, 4], bf16, name="cond_sb")
        nc.sync.dma_start(out=cond_sb, in_=cond_d.rearrange("(a p) -> p a", p=128))
        b1_sb = consts.tile([128, MFT], f32, name="b1_sb")
        nc.scalar.dma_start(out=b1_sb, in_=b1_d.rearrange("(mt p) -> p mt", p=128))
        wkv_sb = consts.tile([128, FT, 2 * D], bf16, name="wkv_sb")
        nc.sync.dma_start(
            out=wkv_sb, in_=wkv_d.rearrange("(kt p) n -> p kt n", p=128)
        )

        anw_f = consts.tile([128, F], f32, name="anw_f")
        nc.scalar.dma_start(out=anw_f, in_=bcast_row(lnv_d[2:3, :]))
        anb_f = consts.tile([128, F], f32, name="anb_f")
        nc.scalar.dma_start(out=anb_f, in_=bcast_row(lnv_d[3:4, :]))
        anw_bc = consts.tile([128, F], bf16, name="anw_bc")
        nc.vector.tensor_copy(out=anw_bc, in_=anw_f)
        anb_bc = consts.tile([128, F], bf16, name="anb_bc")
        nc.vector.tensor_copy(out=anb_bc, in_=anb_f)
        # bias rows for the K=1 psum-fold matmuls
        wob_row = consts.tile([1, F], f32, name="wob_row")
        nc.scalar.dma_start(out=wob_row, in_=wob_d[0:1, :])
        b2_row = consts.tile([1, F], f32, name="b2_row")
        nc.scalar.dma_start(out=b2_row, in_=b2_d[0:1, :])
        if V_BIAS == "dve":
            wob_bc = consts.tile([128, F], f32, name="wob_bc")
            nc.scalar.dma_start(out=wob_bc, in_=bcast_row(wob_d[0:1, :]))
            b2_bc = consts.tile([128, F], f32, name="b2_bc")
            nc.scalar.dma_start(out=b2_bc, in_=bcast_row(b2_d[0:1, :]))

        # ---------------- phase 0: modulation vectors ----------------
        # modv = cond @ [gA | bA | gF | bF] + modb  -> [1, 4F] fp32, then
        # Wa = amod_nw*(1+gA), Ba = amod_nb*(1+gA)+bA (same for fmod),
        # PE-broadcast to [128, F] bf16 tiles. The four 512-col slices of
        # each half are column-packed onto distinct PE col-groups.
        cm_hera = tc.tile_pool(name="hera", bufs=1)
        hera = cm_hera.__enter__()
        cm_modtmp = tc.tile_pool(name="modtmp", bufs=1)
        modtmp = cm_modtmp.__enter__()

        lnr = {}
        for r in (0, 1, 4, 5):  # amod_nw/nb, fmod_nw/nb rows at partition 0
            lnr[r] = modtmp.tile([1, F], f32, name=f"lnr{r}")
            nc.scalar.dma_start(out=lnr[r], in_=lnv_d[r : r + 1, :])
        modb_sb = modtmp.tile([1, 4 * F], f32, name="modb_sb")
        nc.scalar.dma_start(out=modb_sb, in_=modb_d.rearrange("(a f) -> a f", a=1))
        modv = modtmp.tile([1, 4 * F], f32, name="modv")
        for grp in range(2):
            wm_tiles = []
            for chd in range(4):
                wm = modtmp.tile(
                    [128, 2048], bf16, tag="wm", bufs=2, name=f"wm{grp}_{chd}"
                )
                nc.sync.dma_start(
                    out=wm,
                    in_=wmod_d[
                        chd * 128 : (chd + 1) * 128, grp * 2048 : (grp + 1) * 2048
                    ],
                )
                wm_tiles.append(wm)
            pms = [
                psF.tile([128, 512], f32, tag="pmod", bufs=4, name=f"pm{grp}_{j}")
                for j in range(4)
            ]
            for chd in range(4):
                for j in range(4):
                    nc.tensor.matmul(
                        pms[j][0:1, :],
                        cond_sb[:, chd : chd + 1],
                        wm_tiles[chd][:, j * 512 : (j + 1) * 512],
                        start=(chd == 0),
                        stop=(chd == 3),
                    )
            for j in range(4):
                nb = grp * 4 + j
                nc.vector.tensor_add(
                    out=modv[:, nb * 512 : (nb + 1) * 512],
                    in0=pms[j][0:1, :],
                    in1=modb_sb[:, nb * 512 : (nb + 1) * 512],
                )

        tmpv = modtmp.tile([1, F], f32, name="tmpv")
        bc = {}
        modv16 = modtmp.tile([1, 4 * F], f16, name="modv16")

        def finalize_mod(g_off, b_off, nw_row, nb_row, w_name, b_name):
            g_sl = modv[:, g_off : g_off + F]
            b_sl = modv[:, b_off : b_off + F]
            nc.scalar.add(out=g_sl, in_=g_sl, add=1.0)
            nc.vector.tensor_mul(out=tmpv, in0=g_sl, in1=lnr[nb_row])
            with nc.allow_low_precision(reason="f16 staging for PE broadcast"):
                nc.vector.tensor_add(
                    out=modv16[:, b_off : b_off + F], in0=tmpv, in1=b_sl
                )
                nc.vector.tensor_mul(
                    out=modv16[:, g_off : g_off + F], in0=g_sl, in1=lnr[nw_row]
                )
            for off, nm in ((g_off, w_name), (b_off, b_name)):
                bt = consts.tile([128, F], bf16, name=nm)
                for hf in range(2):
                    pb = psF.tile([128, 512], f32, tag="pmod", bufs=4, name="pbc")
                    nc.tensor.matmul(
                        pb,
                        ones16,
                        modv16[:, off + hf * 512 : off + (hf + 1) * 512],
                        start=True,
                        stop=True,
                    )
                    nc.scalar.activation(
                        bt[:, hf * 512 : (hf + 1) * 512], pb, AF.Copy
                    )
                bc[nm] = bt

        finalize_mod(0, F, 0, 1, "Wa_bc", "Ba_bc")

        # ---------------- phase 1: adaLN-1 + attn-LN + kv + gather ----------------

        hT = [
            persist.tile([128, R], bf16, tag=f"hT{ft}", name=f"hT{ft}")
            for ft in range(FT)
        ]
        h_res = [hera.tile([128, F], bf16, name=f"h{rb}") for rb in range(RB)]
        kvT_sb = hera.tile([128, R], bf16, name="kvT_sb")

        kv_bounce = [dramp.tile([2 * D, 256], bf16, name=f"kvb{c}") for c in range(2)]
        kv_all = [dramp.tile([4 * 2 * D, 256], bf16, name=f"kva{c}") for c in range(2)]

        def layer_norm_stats(src):
            stats = work.tile([128, 2, 6], f32, tag="stats", name="stats")
            for sg in range(2):
                nc.vector.bn_stats(
                    out=stats[:, sg, :], in_=src[:, sg * 512 : (sg + 1) * 512]
                )
            mv = work.tile([128, 2], f32, tag="mv", name="mv")
            nc.vector.bn_aggr(out=mv, in_=stats)
            rstd = work.tile([128, 1], f32, tag="rstd", name="rstd")
            nc.scalar.activation(
                out=rstd, in_=mv[:, 1:2], func=AF.Sqrt, bias=epst, scale=1.0
            )
            nc.vector.reciprocal(out=rstd, in_=rstd)
            return mv, rstd

        def transpose_to_h2(hsrc_bf, rb, tpool):
            for ft in range(FT):
                pt = tpool.tile([128, 128], bf16, tag="pt", bufs=2, name="ptt2")
                nc.tensor.transpose(
                    pt, hsrc_bf[:, ft * 128 : (ft + 1) * 128], ident
                )
                nc.scalar.activation(
                    out=h2all[:, ft, rb * 128 : (rb + 1) * 128],
                    in_=pt,
                    func=AF.Copy,
                )

        def transpose_to(hsrc_bf, hT_tiles, rb, tpool):
            for ft in range(FT):
                pt = tpool.tile([128, 128], bf16, tag="pt", bufs=2, name="ptt")
                nc.tensor.transpose(
                    pt, hsrc_bf[:, ft * 128 : (ft + 1) * 128], ident
                )
                nc.scalar.activation(
                    out=hT_tiles[ft][:, rb * 128 : (rb + 1) * 128],
                    in_=pt,
                    func=AF.Copy,
                )

        with nc.named_scope("p1_ln"):
            xns = []
            for rb in range(RB):
                x_rb = work.tile([128, F], f32, tag="x", bufs=3, name="x_rb")
                nc.sync.dma_start(out=x_rb, in_=x_d[rb * 128 : (rb + 1) * 128, :])
                mv, rstd = layer_norm_stats(x_rb)
                xn = hera.tile([128, F], bf16, name=f"xnp{rb}")
                nc.vector.tensor_scalar(
                    out=xn,
                    in0=x_rb,
                    scalar1=mv[:, 0:1],
                    scalar2=rstd,
                    op0=OP.subtract,
                    op1=OP.mult,
                )
                xns.append(xn)
            for rb in range(RB):
                h0 = work.tile([128, F], bf16, tag="h0", name="h0")
                nc.vector.tensor_mul(out=h0, in0=xns[rb], in1=bc["Wa_bc"])
                nc.vector.tensor_add(out=h_res[rb], in0=h0, in1=bc["Ba_bc"])
                # attn-LN
                mv2, rstd2 = layer_norm_stats(h_res[rb])
                xn2 = work.tile([128, F], bf16, tag="xn", name="xn2")
                nc.vector.tensor_scalar(
                    out=xn2,
                    in0=h_res[rb],
                    scalar1=mv2[:, 0:1],
                    scalar2=rstd2,
                    op0=OP.subtract,
                    op1=OP.mult,
                )
                hn1 = work.tile([128, F], bf16, tag="h0", name="hn1")
                nc.vector.tensor_mul(out=hn1, in0=xn2, in1=anw_bc)
                hn_bf = work.tile([128, F], bf16, tag="hnbf", name="hn_bf")
                nc.vector.tensor_add(out=hn_bf, in0=hn1, in1=anb_bc)
                transpose_to(hn_bf, hT, rb, psF)
                # kv projection for this row block
                pkv = psF.tile([128, 512], f32, tag="pkq", bufs=2, name="pkv")
                for kt in range(FT):
                    nc.tensor.matmul(
                        pkv[:, 0:128],
                        wkv_sb[:, kt, :],
                        hT[kt][:, rb * 128 : (rb + 1) * 128],
                        start=(kt == 0),
                        stop=(kt == FT - 1),
                    )
                nc.scalar.activation(
                    out=kvT_sb[:, rb * 128 : (rb + 1) * 128],
                    in_=pkv[:, 0:128],
                    func=AF.Copy,
                )
                if rb % 2 == 1:
                    c = rb // 2
                    with nc.named_scope(f"gather{c}"):
                        nc.sync.dma_start(
                            out=kv_bounce[c], in_=kvT_sb[:, c * 256 : (c + 1) * 256]
                        )
                        nc.gpsimd.collective_compute(
                            "AllGather",
                            OP.bypass,
                            replica_groups=groups,
                            ins=[kv_bounce[c][:, :]],
                            outs=[kv_all[c][:, :]],
                        )

        finalize_mod(2 * F, 3 * F, 4, 5, "Wf_bc", "Bf_bc")
        cm_modtmp.__exit__(None, None, None)
        cm_aera = tc.tile_pool(name="aera", bufs=1)
        aera = cm_aera.__enter__()

        # ---------------- phase 2: q proj ----------------
        qT = [aera.tile([128, R], bf16, name=f"qT{mt}") for mt in range(MT)]
        with nc.named_scope("qproj"):
            for mt in range(MT):
                wqblk = wstr.tile(
                    [128, FT * 128], bf16, tag="wqb", bufs=2, name="wqblk"
                )
                nc.sync.dma_start(out=wqblk, in_=wq_d[mt])
                pq = psF.tile([128, 512], f32, tag="pkq", bufs=2, name="pq")
                for kt in range(FT):
                    nc.tensor.matmul(
                        pq,
                        wqblk[:, kt * 128 : (kt + 1) * 128],
                        hT[kt],
                        start=(kt == 0),
                        stop=(kt == FT - 1),
                    )
                # fold the attention 1/sqrt(D)=0.125 scale into q
                nc.scalar.activation(out=qT[mt], in_=pq, func=AF.Copy, scale=0.125)

        # ---------------- phase 3: kT / v_ext assembly ----------------
        # k^T duplicated into both partition halves so MM1's lhsT can share
        # the rhs (q head slice) base partition for even and odd heads.
        kT = aera.tile([128, T], bf16, name="kT")
        # fp8 DoubleRow PV weights: vd[j] holds 16*v for key tiles (2j, 2j+1)
        # in [Ki, ko, col] layout, col 64 = 16.0 (the softmax-denominator ones
        # column, pre-scaled to keep v in fp8 normal range).
        vd = [aera.tile([128, 2, 80], fp8, name=f"vd{j}") for j in range(KT // 2)]
        with nc.named_scope("asm"):
            for j in range(KT // 2):
                nc.vector.memset(vd[j][:, :, 64:65], 16.0)
            for c in range(2):
                for r in range(4):
                    for hp in (0, 64):
                        nc.sync.dma_start(
                            out=kT[
                                hp : hp + 64, r * 512 + c * 256 : r * 512 + (c + 1) * 256
                            ],
                            in_=kv_all[c][r * 128 : r * 128 + 64, :],
                        )
                    vT_sb = work.tile([64, 256], bf16, tag="vTs", name="vT_sb")
                    nc.sync.dma_start(
                        out=vT_sb, in_=kv_all[c][r * 128 + 64 : (r + 1) * 128, :]
                    )
                    for cc in range(2):
                        kt = r * 4 + c * 2 + cc
                        ptv = psF.tile(
                            [128, 128], bf16, tag="pt", bufs=2, name="ptv"
                        )
                        nc.tensor.matmul(
                            ptv[:, 0:64],
                            vT_sb[:, cc * 128 : (cc + 1) * 128],
                            ident[0:64, 0:64],
                            is_transpose=True,
                        )
                        nc.scalar.activation(
                            out=vd[kt // 2][:, kt % 2, 0:64],
                            in_=ptv[:, 0:64],
                            func=AF.Copy,
                            scale=16.0,
                        )

        # prefetch wo during attention (sync queue)
        if V_OPACK == "dma":
            woc = [aera.tile([128, F], bf16, name=f"woc{m}") for m in range(MT)]
            for m in range(MT):
                nc.sync.dma_start(out=woc[m], in_=wo_d[m * 128 : (m + 1) * 128, :])
        else:
            woc_f = [aera.tile([64, F], bf16, name=f"wocf{k}") for k in range(H)]
            for k in range(H):
                nc.sync.dma_start(
                    out=woc_f[k], in_=wo_d[k * 64 : (k + 1) * 64, :]
                )

        cm_psF.__exit__(None, None, None)
        cm_psAT = tc.tile_pool(name="psAT", bufs=1, space="PSUM")
        psAT = cm_psAT.__enter__()

        # ---------------- phase 4: attention ----------------
        # Per head: MM1 into [128,1024] psum tiles (2 key tiles per bank
        # pair), exp at FD=1024 psum->sbuf bf16, PV with ones-column denom.
        # PE instruction order interleaves MM1(h) with PV(h-1) so the queue
        # head is never blocked on exp.
        if V_OPACK == "dma":
            outTp = [aera.tile([128, R], bf16, name=f"outTp{m}") for m in range(MT)]
        else:
            outT_f = [aera.tile([64, R], bf16, name=f"outTf{k}") for k in range(H)]
        probs_all = {}
        probs_sc = {}

        def emit_mm1(hi, j):
            # scores for key tiles (2j, 2j+1) of head hi; two 1-bank psum
            # tiles exp'd separately into halves of the fp8 probs pair tile
            mt, hp = hi // 2, (hi % 2) * 64
            pr = aera.tile(
                [128, 2, 512], fp8, tag="probs", bufs=20, name=f"pr{hi}_{j}"
            )
            for cc in range(2):
                kt = 2 * j + cc
                sc = psAT.tile(
                    [128, 512], f32, tag="sc", bufs=4, name=f"sc{hi}_{j}_{cc}"
                )
                nc.tensor.matmul(
                    sc,
                    kT[hp : hp + 64, kt * 128 : (kt + 1) * 128],
                    qT[mt][hp : hp + 64, :],
                    start=True,
                    stop=True,
                )
                if hi % 2 == 1 and cc == 1:
                    # Schraudolph fast-exp on DVE: exp(s) ~ bits(A*s + B) as
                    # fp32 (~3% sawtooth, same order as the fp8 probs quant;
                    # softmax averaging washes it out). Offloads ~25% of the
                    # exp work from the saturated ScalarE.
                    ti = work.tile(
                        [128, 512], mybir.dt.int32, tag="ti", name="ti"
                    )
                    with nc.allow_low_precision(reason="approx exp for probs"):
                        nc.vector.tensor_scalar(
                            out=ti,
                            in0=sc,
                            scalar1=12102203.161561485,
                            scalar2=1064866805.0,
                            op0=OP.mult,
                            op1=OP.add,
                        )
                        nc.vector.tensor_copy(
                            out=pr[:, cc, :], in_=ti[:, :].bitcast(f32)
                        )
                else:
                    nc.scalar.activation(out=pr[:, cc, :], in_=sc, func=AF.Exp)
            probs_all[(hi, j)] = pr

        def emit_pv(hi, j, po):
            # fp8 DoubleRow: contracts key tiles 2j and 2j+1 in one matmul
            pr = probs_all.pop((hi, j))
            nc.tensor.matmul(
                po[0:65, :],
                vd[j][:, :, 0:65],
                pr,
                start=(j == 0),
                stop=(j == 7),
                perf_mode=DR,
            )

        def emit_normalize(hi, po):
            # reciprocal of the denominator row, broadcast to 64 partitions
            # via a DRAM bounce (no PE involvement -> no PE queue blocking)
            m, hp = hi // 2, (hi % 2) * 64
            rcp_row = work.tile([128, R], f16, tag="rcp", name="rcp_row")
            with nc.allow_low_precision(reason="f16 softmax denom"):
                nc.vector.reciprocal(out=rcp_row[64:65, :], in_=po[64:65, :])
            rden = dramp.tile([1, R], f16, tag="rden", bufs=2, name=f"rden{hi}")
            nc.sync.dma_start(out=rden, in_=rcp_row[64:65, :])
            rcpb = work.tile([64, R], f16, tag="rcpb", name="rcpb")
            nc.sync.dma_start(out=rcpb, in_=bcast_row(rden[0:1, :], nparts=64))
            if hp == 0:
                nc.vector.tensor_mul(
                    out=outTp[m][0:64, :], in0=po[0:64, :], in1=rcpb
                )
            else:
                # DVE cannot shift partitions; stage at base 0 then DMA up
                oT = work.tile([64, R], bf16, tag="oT", name="oT")
                nc.vector.tensor_mul(out=oT, in0=po[0:64, :], in1=rcpb)
                nc.sync.dma_start(out=outTp[m][64:128, :], in_=oT)

        JSEQ = [0, 2, 4, 6, 1, 3, 5, 7]  # chunk-0 key tiles first
        with nc.named_scope("attn"):
            po_t = {}
            for j in JSEQ:
                emit_mm1(0, j)
                emit_mm1(1, j)
            for m in range(MT):
                h0, h1 = 2 * m, 2 * m + 1
                if m > 0:
                    emit_normalize(h0 - 2, po_t.pop(h0 - 2))
                    emit_normalize(h1 - 2, po_t.pop(h1 - 2))
                for hi in (h0, h1):
                    po_t[hi] = psAT.tile(
                        [128, 512], f32, tag="po", bufs=4, name=f"po{hi}"
                    )
                for j in JSEQ:
                    emit_pv(h0, j, po_t[h0])
                    emit_pv(h1, j, po_t[h1])
                    if m + 1 < MT:
                        emit_mm1(h0 + 2, j)
                        emit_mm1(h1 + 2, j)
            emit_normalize(H - 2, po_t.pop(H - 2))
            emit_normalize(H - 1, po_t.pop(H - 1))

        cm_psAT.__exit__(None, None, None)
        cm_psO = tc.tile_pool(name="psO", bufs=1, space="PSUM")
        psO = cm_psO.__enter__()

        # ---------------- phase 5: out proj (head pairs) + residual -> x1 ----------------
        x1 = [persist.tile([128, F], f32, name=f"x1_{rt}") for rt in range(RB)]
        with nc.named_scope("oproj"):
            px1 = {}
            for rt in range(RB):
                px1[rt] = psO.tile([128, F], f32, tag="px1", bufs=4, name=f"px1_{rt}")
            nchunk = MT if V_OPACK == "dma" else H
            for m in range(nchunk):
                for rt in range(RB):
                    for nh in range(2):
                        if V_OPACK == "dma":
                            lhsT = outTp[m][:, rt * 128 : (rt + 1) * 128]
                            rhs = woc[m][:, nh * 512 : (nh + 1) * 512]
                        else:
                            lhsT = outT_f[m][:, rt * 128 : (rt + 1) * 128]
                            rhs = woc_f[m][:, nh * 512 : (nh + 1) * 512]
                        nc.tensor.matmul(
                            px1[rt][:, nh * 512 : (nh + 1) * 512],
                            lhsT,
                            rhs,
                            start=(m == 0),
                            stop=(V_BIAS == "dve" and m == nchunk - 1),
                        )
            for rt in range(RB):
                if V_BIAS == "mm":
                    for nh in range(2):
                        # fold wo_b into the accumulator: += ones^T(rows) x wob
                        nc.tensor.matmul(
                            px1[rt][:, nh * 512 : (nh + 1) * 512],
                            ones_b,
                            wob_row[:, nh * 512 : (nh + 1) * 512],
                            start=False,
                            stop=True,
                        )
                nc.vector.tensor_add(out=x1[rt], in0=px1[rt], in1=h_res[rt])
                if V_BIAS == "dve":
                    nc.vector.tensor_add(out=x1[rt], in0=x1[rt], in1=wob_bc)

        cm_aera.__exit__(None, None, None)
        cm_hera.__exit__(None, None, None)

        cm_psO.__exit__(None, None, None)
        cm_psM = tc.tile_pool(name="psM", bufs=1, space="PSUM")
        psM = cm_psM.__enter__()

        # ---------------- phase 6: adaLN-2 + transpose ----------------
        h2all = persist.tile([128, FT, R], fp8 if V_MLP8 else bf16, name="h2all")
        with nc.named_scope("aln2"):
            for rt in range(RB):
                mv, rstd = layer_norm_stats(x1[rt])
                xn = work.tile([128, F], bf16, tag="xn", name="xn3")
                nc.vector.tensor_scalar(
                    out=xn,
                    in0=x1[rt],
                    scalar1=mv[:, 0:1],
                    scalar2=rstd,
                    op0=OP.subtract,
                    op1=OP.mult,
                )
                h21 = work.tile([128, F], bf16, tag="h0", name="h21")
                nc.vector.tensor_mul(out=h21, in0=xn, in1=bc["Wf_bc"])
                h2_bf = work.tile([128, F], bf16, tag="hnbf", name="h2_bf")
                nc.vector.tensor_add(out=h2_bf, in0=h21, in1=bc["Bf_bc"])
                transpose_to_h2(h2_bf, rt, psM)

        # ---------------- phase 7+8: mlp1 + gelu, mlp2 interleaved ----------------
        cm_mlp = tc.tile_pool(name="mlpera", bufs=1)
        mlpera = cm_mlp.__enter__()
        g1all = mlpera.tile([128, MFT, R], fp8 if V_MLP8 else bf16, name="g1all")
        pf = {}
        with nc.named_scope("mlp"):
            if V_MLP8:
                w2f = [
                    mlpera.tile([128, 2, F], fp8, name=f"w2f{k}")
                    for k in range(MFT // 2)
                ]
                for k in range(MFT // 2):
                    eng = nc.gpsimd if k % 2 == 0 else nc.scalar
                    eng.dma_start(out=w2f[k], in_=w2_d[k])
            for rt in range(RB):
                pf[rt] = psM.tile([128, 512], f32, tag="pf", bufs=4, name=f"pf{rt}")

            def mlp2_chunk8(k, fh, pfd):
                # fp8 DR mlp2 accumulation for hidden pair k, F-half fh
                for rt in range(RB):
                    nc.tensor.matmul(
                        pfd[rt],
                        g1all[:, 2 * k : 2 * k + 2, rt * 128 : (rt + 1) * 128],
                        w2f[k][:, :, fh * 512 : (fh + 1) * 512],
                        start=(k == 0),
                        stop=False,
                        perf_mode=DR,
                    )

            def mlp2_chunk16(mt, fh, pfd):
                # bf16 mlp2 accumulation for hidden tile mt
                w2c = wstr.tile([128, 512], bf16, tag="w2c", bufs=3, name="w2c")
                eng = nc.gpsimd if mt % 2 == 0 else nc.scalar
                eng.dma_start(
                    out=w2c,
                    in_=w2_d[mt * 128 : (mt + 1) * 128, fh * 512 : (fh + 1) * 512],
                )
                for rt in range(RB):
                    nc.tensor.matmul(
                        pfd[rt],
                        g1all[:, mt, rt * 128 : (rt + 1) * 128],
                        w2c,
                        start=(mt == 0),
                        stop=False,
                    )

            for mt in range(MFT):
                if V_MLP8:
                    w1blk = wstr.tile(
                        [128, FT // 2, 2, 128], fp8, tag="w1b", bufs=3, name="w1blk"
                    )
                    eng = nc.sync if mt % 2 == 0 else nc.scalar
                    eng.dma_start(
                        out=w1blk,
                        in_=w1_d[mt].rearrange(
                            "p (j two m) -> p j two m", j=FT // 2, two=2
                        ),
                    )
                else:
                    w1blk = wstr.tile(
                        [128, FT * 128], bf16, tag="w1b", bufs=3, name="w1blk"
                    )
                    eng = nc.sync if mt % 2 == 0 else nc.scalar
                    eng.dma_start(out=w1blk, in_=w1_d[mt])
                pg = psM.tile([128, 512], f32, tag="pg", bufs=2, name="pg")
                if V_MLP8:
                    for j in range(FT // 2):
                        nc.tensor.matmul(
                            pg,
                            w1blk[:, j],
                            h2all[:, 2 * j : 2 * j + 2, :],
                            start=(j == 0),
                            stop=(j == FT // 2 - 1),
                            perf_mode=DR,
                        )
                else:
                    for kt in range(FT):
                        nc.tensor.matmul(
                            pg,
                            w1blk[:, kt * 128 : (kt + 1) * 128],
                            h2all[:, kt, :],
                            start=(kt == 0),
                            stop=(kt == FT - 1),
                        )
                nc.scalar.activation(
                    out=g1all[:, mt, :],
                    in_=pg,
                    func=AF.Gelu,
                    bias=b1_sb[:, mt : mt + 1],
                    scale=(1.0 / 256.0) if V_MLP8 else 1.0,
                )
                if V_MLP8:
                    if mt % 2 == 1 and mt > 1:
                        mlp2_chunk8((mt - 2) // 2, 0, pf)
                elif mt > 0:
                    mlp2_chunk16(mt - 1, 0, pf)
            if V_MLP8:
                mlp2_chunk8(MFT // 2 - 1, 0, pf)
            else:
                mlp2_chunk16(MFT - 1, 0, pf)
            oscale = (1.0 / 256.0) if V_MLP8 else 1.0
            for rt in range(RB):
                nc.tensor.matmul(
                    pf[rt], ones_b, b2_row[:, 0:512], start=False, stop=True
                )
                yh1 = work.tile([128, 512], f32, tag="yh1", name="yh1")
                nc.scalar.activation(out=yh1, in_=pf[rt], func=AF.Copy, scale=oscale)
                yh = work.tile([128, 512], f32, tag="yh", name="yh")
                nc.vector.tensor_add(out=yh, in0=yh1, in1=x1[rt][:, 0:512])
                nc.sync.dma_start(out=y_d[rt * 128 : (rt + 1) * 128, 0:512], in_=yh)
            # second half of mlp2
            pf2 = {}
            for rt in range(RB):
                pf2[rt] = psM.tile([128, 512], f32, tag="pf", bufs=4, name=f"pf2{rt}")
            if V_MLP8:
                for k in range(MFT // 2):
                    mlp2_chunk8(k, 1, pf2)
            else:
                for mt in range(MFT):
                    mlp2_chunk16(mt, 1, pf2)
            for rt in range(RB):
                nc.tensor.matmul(
                    pf2[rt], ones_b, b2_row[:, 512:1024], start=False, stop=True
                )
                yh1 = work.tile([128, 512], f32, tag="yh1", name="yh1b")
                nc.scalar.activation(out=yh1, in_=pf2[rt], func=AF.Copy, scale=oscale)
                yh = work.tile([128, 512], f32, tag="yh", name="yh2")
                nc.vector.tensor_add(out=yh, in0=yh1, in1=x1[rt][:, 512:1024])
                nc.sync.dma_start(
                    out=y_d[rt * 128 : (rt + 1) * 128, 512:1024], in_=yh
                )
        cm_mlp.__exit__(None, None, None)
        cm_psM.__exit__(None, None, None)

    nc.compile()
    return nc


def _prep_in_maps(inputs):
    f32 = np.float32
    wmod = np.concatenate(
        [inputs["amod_gw"], inputs["amod_bw"], inputs["fmod_gw"], inputs["fmod_bw"]],
        axis=1,
    ).astype(BF16)
    modb = np.concatenate(
        [inputs["amod_gb"], inputs["amod_bb"], inputs["fmod_gb"], inputs["fmod_bb"]]
    ).astype(f32)
    lnvec = np.stack(
        [
            inputs["amod_nw"],
            inputs["amod_nb"],
            inputs["attn_nw"],
            inputs["attn_nb"],
            inputs["fmod_nw"],
            inputs["fmod_nb"],
        ]
    ).astype(f32)
    wq_t = np.ascontiguousarray(
        np.asarray(inputs["wq"]).astype(BF16).reshape(FT, 128, MT, 128)
        .transpose(2, 1, 0, 3).reshape(MT, 128, FT * 128)
    )
    FP8 = ml_dtypes.float8_e4m3
    if int(os.environ.get("V_MLP8", "1")):
        # w1 * 256 in DoubleRow layout [mt, i, (j two m)]
        w1_t = np.ascontiguousarray(
            np.clip(np.asarray(inputs["w1"], np.float32) * 256.0, -240, 240)
            .reshape(FT // 2, 2, 128, MFT, 128)
            .transpose(3, 2, 0, 1, 4).reshape(MFT, 128, FT * 128)
        ).astype(FP8)
        # w2 * 256 in DoubleRow rhs layout [k, i, (two f)]
        w2_t = np.ascontiguousarray(
            np.clip(np.asarray(inputs["w2"], np.float32) * 256.0, -240, 240)
            .reshape(MFT // 2, 2, 128, F)
            .transpose(0, 2, 1, 3).reshape(MFT // 2, 128, 2 * F)
        ).astype(FP8)
        b2_t = np.asarray(inputs["b2"]).astype(f32).reshape(1, F) * 256.0
    else:
        w1_t = np.ascontiguousarray(
            np.asarray(inputs["w1"]).astype(BF16).reshape(FT, 128, MFT, 128)
            .transpose(2, 1, 0, 3).reshape(MFT, 128, FT * 128)
        )
        w2_t = np.asarray(inputs["w2"]).astype(BF16)
        b2_t = np.asarray(inputs["b2"]).astype(f32).reshape(1, F)
    shared = dict(
        wmod=wmod,
        modb=modb,
        lnvec=lnvec,
        wq=wq_t,
        wkv=np.asarray(inputs["wkv"]).astype(BF16),
        wo=np.asarray(inputs["wo"]).astype(BF16),
        wo_bias=np.asarray(inputs["wo_b"]).astype(f32).reshape(1, F),
        w1=w1_t,
        b1=np.asarray(inputs["b1"]).astype(f32),
        w2=w2_t,
        b2=b2_t,
    )
    x = np.asarray(inputs["x"]).astype(f32)
    cond = np.asarray(inputs["cond"]).astype(BF16)
    in_maps = []
    for c in range(NCORES):
        b, r0 = c // 4, (c % 4) * R
        m = dict(shared)
        m["x"] = np.ascontiguousarray(x[b, r0 : r0 + R, :])
        m["cond"] = np.ascontiguousarray(cond[b])
        in_maps.append(m)
    return in_maps


def _run(inputs, trace=False, tmpdir=None):
    from concourse.bass_utils import run_bass_kernel_spmd

    if "nc" not in _CACHE:
        _CACHE["nc"] = _build_nc()
    nc = _CACHE["nc"]
    in_maps = _prep_in_maps(inputs)
    res = run_bass_kernel_spmd(
        nc, in_maps, core_ids=list(range(NCORES)), trace=trace, tmpdir=tmpdir
    )
    y = np.empty((B, T, F), np.float32)
    for c in range(NCORES):
        b, r0 = c // 4, (c % 4) * R
        y[b, r0 : r0 + R, :] = res.results[c]["y"]
    return y, res


def kernel(**inputs) -> np.ndarray:
    y, _ = _run(inputs, trace=False)
    return y


if __name__ == "__main__":
    _build_nc()
    print("build OK")


# revision 29
# speedup vs baseline: 1.0551x; 1.0309x over previous
"""DiT block kernel for 8x Trainium2 NeuronCores (Bass/Tile).

Sharding: row-parallel over the flattened (B,T)=4096 rows; 512 rows/core.
Cores 0-3 handle batch 0, cores 4-7 batch 1. MQA K/V is computed per-shard
and AllGather'd (in two row-chunks, launched as soon as the rows are
LayerNormed) within each 4-core batch group. Weights are replicated and
cast to bf16; LN stats and residual accumulation stay fp32, the modulated
residual h is carried in bf16.

Performance notes (vs the first working version):
  - modulation matmuls (M=1) are column-packed 4-wide via tile_position
  - LN affine chain runs in bf16 (DVE 2x modes), adds on DVE not GpSimd
    (GpSimd stays empty so the AllGather triggers fire immediately)
  - attention: scores psum tiles hold 2 key-tiles so exp runs at FD=1024;
    MM1 of head h+1 is interleaved with PV of head h instruction-by-
    instruction so the PE never sits behind a blocked queue head; softmax
    denominators use the ones-column trick + reciprocal_approx_fast
  - out-projection packs head pairs (K=128) and folds wo_b into the psum
    accumulation via a K=1 ones-row matmul (same for b2 in mlp2)
  - weight streams issue from sync/scalar/gpsimd queues, never stealing
    ScalarE time during attention (exp is the attention bottleneck)
"""

import os
import sys

sys.path.insert(0, "/opt/trn_rl_repo")

import numpy as np
import ml_dtypes

BF16 = ml_dtypes.bfloat16

B, T, F, H, D, M, C = 2, 2048, 1024, 16, 64, 4, 512
NCORES = 8
R = (B * T) // NCORES  # 512 rows per core
RB = R // 128  # 4 row blocks
FT = F // 128  # 8 feature tiles
MT = (H * D) // 128  # 8 head-pair tiles
MFT = (M * F) // 128  # 32 mlp hidden tiles
KT = T // 128  # 16 key tiles
EPS = 1e-5

_CACHE = {}

# HW-bisect feature flags (baseline-proven defaults)
V_RECIP = os.environ.get("V_RECIP", "exact")   # exact | fast (fast is broken on HW)
V_EXP = int(os.environ.get("V_EXP", "1024"))   # 512 | 1024
V_BIAS = os.environ.get("V_BIAS", "mm")        # dve | mm
V_OPACK = os.environ.get("V_OPACK", "dma")     # flat | dma
V_MLP8 = int(os.environ.get("V_MLP8", "0"))    # 1 = fp8 DoubleRow MLP, 0 = bf16


def _build_nc():
    import concourse.bass as bass
    import concourse.tile as tile
    from concourse import bacc, mybir
    from concourse.masks import make_identity
    from contextlib import ExitStack

    f32 = mybir.dt.float32
    f16 = mybir.dt.float16
    bf16 = mybir.dt.bfloat16
    fp8 = mybir.dt.float8e4
    DR = mybir.MatmulPerfMode.DoubleRow
    AF = mybir.ActivationFunctionType
    OP = mybir.AluOpType

    nc = bacc.Bacc(
        "TRN2",
        target_bir_lowering=False,
        debug=False,
        enable_asserts=False,
        num_devices=NCORES,
    )

    def dram(name, shape, dt, **kw):
        return nc.dram_tensor(name, shape, dt, **kw).ap()

    x_d = dram("x", [R, F], f32, kind="ExternalInput")
    cond_d = dram("cond", [C], bf16, kind="ExternalInput")
    wmod_d = dram("wmod", [C, 4 * F], bf16, kind="ExternalInput")
    modb_d = dram("modb", [4 * F], f32, kind="ExternalInput")
    lnv_d = dram("lnvec", [6, F], f32, kind="ExternalInput")
    wq_d = dram("wq", [MT, 128, FT * 128], bf16, kind="ExternalInput")
    wkv_d = dram("wkv", [F, 2 * D], bf16, kind="ExternalInput")
    wo_d = dram("wo", [H * D, F], bf16, kind="ExternalInput")
    wob_d = dram("wo_bias", [1, F], f32, kind="ExternalInput")
    if V_MLP8:
        w1_d = dram("w1", [MFT, 128, FT * 128], fp8, kind="ExternalInput")
        w2_d = dram("w2", [MFT // 2, 128, 2 * F], fp8, kind="ExternalInput")
    else:
        w1_d = dram("w1", [MFT, 128, FT * 128], bf16, kind="ExternalInput")
        w2_d = dram("w2", [M * F, F], bf16, kind="ExternalInput")
    b1_d = dram("b1", [M * F], f32, kind="ExternalInput")
    b2_d = dram("b2", [1, F], f32, kind="ExternalInput")
    y_d = dram("y", [R, F], f32, kind="ExternalOutput")

    groups = [[0, 1, 2, 3], [4, 5, 6, 7]]

    def bcast_row(ap_row, nparts=128):
        # [1, n] DRAM AP -> partition-broadcast [nparts, n]
        return bass.AP(
            tensor=ap_row.tensor,
            offset=ap_row.offset,
            ap=[[0, nparts]] + list(ap_row.ap[-1:]),
        )

    with tile.TileContext(nc) as tc, ExitStack() as ctx:
        consts = ctx.enter_context(tc.tile_pool(name="consts", bufs=1))
        work = ctx.enter_context(tc.tile_pool(name="work", bufs=2))
        persist = ctx.enter_context(tc.tile_pool(name="persist", bufs=1))
        wstr = ctx.enter_context(tc.tile_pool(name="wstr", bufs=3))
        dramp = ctx.enter_context(tc.tile_pool(name="dramp", bufs=1, space="DRAM"))
        # phase-scoped PSUM pools (8 banks each era)
        cm_psF = tc.tile_pool(name="psF", bufs=1, space="PSUM")
        psF = cm_psF.__enter__()

        # ---------------- constants ----------------
        ident = consts.tile([128, 128], bf16, name="ident")
        make_identity(nc, ident)
        ones16 = consts.tile([1, 128], f16, name="ones16")
        nc.vector.memset(ones16, 1.0)
        # fp32 ones row at partition 64 (softmax denom broadcast) and at
        # partition 0 (bias fold into psum accumulators)
        ones_dn = consts.tile([128, 64], f32, name="ones_dn")
        nc.vector.memset(ones_dn[64:65, :], 1.0)
        ones_dn16 = consts.tile([128, 64], f16, name="ones_dn16")
        nc.vector.memset(ones_dn16[64:65, :], 1.0)
        ones_b = consts.tile([1, 128], f32, name="ones_b")
        nc.vector.memset(ones_b, 1.0)
        epst = consts.tile([128, 1], f32, name="epst")
        nc.vector.memset(epst, EPS)

        cond_sb = consts.tile([128, 4], bf16, name="cond_sb")
        nc.sync.dma_start(out=cond_sb, in_=cond_d.rearrange("(a p) -> p a", p=128))
        b1_sb = consts.tile([128, MFT], f32, name="b1_sb")
        nc.scalar.dma_start(out=b1_sb, in_=b1_d.rearrange("(mt p) -> p mt", p=128))
        wkv_sb = consts.tile([128, FT, 2 * D], bf16, name="wkv_sb")
        nc.sync.dma_start(
            out=wkv_sb, in_=wkv_d.rearrange("(kt p) n -> p kt n", p=128)
        )

        anw_f = consts.tile([128, F], f32, name="anw_f")
        nc.scalar.dma_start(out=anw_f, in_=bcast_row(lnv_d[2:3, :]))
        anb_f = consts.tile([128, F], f32, name="anb_f")
        nc.scalar.dma_start(out=anb_f, in_=bcast_row(lnv_d[3:4, :]))
        anw_bc = consts.tile([128, F], bf16, name="anw_bc")
        nc.vector.tensor_copy(out=anw_bc, in_=anw_f)
        anb_bc = consts.tile([128, F], bf16, name="anb_bc")
        nc.vector.tensor_copy(out=anb_bc, in_=anb_f)
        # bias rows for the K=1 psum-fold matmuls
        wob_row = consts.tile([1, F], f32, name="wob_row")
        nc.scalar.dma_start(out=wob_row, in_=wob_d[0:1, :])
        b2_row = consts.tile([1, F], f32, name="b2_row")
        nc.scalar.dma_start(out=b2_row, in_=b2_d[0:1, :])
        if V_BIAS == "dve":
            wob_bc = consts.tile([128, F], f32, name="wob_bc")
            nc.scalar.dma_start(out=wob_bc, in_=bcast_row(wob_d[0:1, :]))
            b2_bc = consts.tile([128, F], f32, name="b2_bc")
            nc.scalar.dma_start(out=b2_bc, in_=bcast_row(b2_d[0:1, :]))

        # ---------------- phase 0: modulation vectors ----------------
        # modv = cond @ [gA | bA | gF | bF] + modb  -> [1, 4F] fp32, then
        # Wa = amod_nw*(1+gA), Ba = amod_nb*(1+gA)+bA (same for fmod),
        # PE-broadcast to [128, F] bf16 tiles. The four 512-col slices of
        # each half are column-packed onto distinct PE col-groups.
        cm_hera = tc.tile_pool(name="hera", bufs=1)
        hera = cm_hera.__enter__()
        cm_modtmp = tc.tile_pool(name="modtmp", bufs=1)
        modtmp = cm_modtmp.__enter__()

        lnr = {}
        for r in (0, 1, 4, 5):  # amod_nw/nb, fmod_nw/nb rows at partition 0
            lnr[r] = modtmp.tile([1, F], f32, name=f"lnr{r}")
            nc.scalar.dma_start(out=lnr[r], in_=lnv_d[r : r + 1, :])
        modb_sb = modtmp.tile([1, 4 * F], f32, name="modb_sb")
        nc.scalar.dma_start(out=modb_sb, in_=modb_d.rearrange("(a f) -> a f", a=1))
        modv = modtmp.tile([1, 4 * F], f32, name="modv")
        for grp in range(2):
            wm_tiles = []
            for chd in range(4):
                wm = modtmp.tile(
                    [128, 2048], bf16, tag="wm", bufs=2, name=f"wm{grp}_{chd}"
                )
                nc.sync.dma_start(
                    out=wm,
                    in_=wmod_d[
                        chd * 128 : (chd + 1) * 128, grp * 2048 : (grp + 1) * 2048
                    ],
                )
                wm_tiles.append(wm)
            pms = [
                psF.tile([128, 512], f32, tag="pmod", bufs=4, name=f"pm{grp}_{j}")
                for j in range(4)
            ]
            for chd in range(4):
                for j in range(4):
                    nc.tensor.matmul(
                        pms[j][0:1, :],
                        cond_sb[:, chd : chd + 1],
                        wm_tiles[chd][:, j * 512 : (j + 1) * 512],
                        start=(chd == 0),
                        stop=(chd == 3),
                    )
            for j in range(4):
                nb = grp * 4 + j
                nc.vector.tensor_add(
                    out=modv[:, nb * 512 : (nb + 1) * 512],
                    in0=pms[j][0:1, :],
                    in1=modb_sb[:, nb * 512 : (nb + 1) * 512],
                )

        tmpv = modtmp.tile([1, F], f32, name="tmpv")
        bc = {}
        modv16 = modtmp.tile([1, 4 * F], f16, name="modv16")

        def finalize_mod(g_off, b_off, nw_row, nb_row, w_name, b_name):
            g_sl = modv[:, g_off : g_off + F]
            b_sl = modv[:, b_off : b_off + F]
            nc.scalar.add(out=g_sl, in_=g_sl, add=1.0)
            nc.vector.tensor_mul(out=tmpv, in0=g_sl, in1=lnr[nb_row])
            with nc.allow_low_precision(reason="f16 staging for PE broadcast"):
                nc.vector.tensor_add(
                    out=modv16[:, b_off : b_off + F], in0=tmpv, in1=b_sl
                )
                nc.vector.tensor_mul(
                    out=modv16[:, g_off : g_off + F], in0=g_sl, in1=lnr[nw_row]
                )
            for off, nm in ((g_off, w_name), (b_off, b_name)):
                bt = consts.tile([128, F], bf16, name=nm)
                for hf in range(2):
                    pb = psF.tile([128, 512], f32, tag="pmod", bufs=4, name="pbc")
                    nc.tensor.matmul(
                        pb,
                        ones16,
                        modv16[:, off + hf * 512 : off + (hf + 1) * 512],
                        start=True,
                        stop=True,
                    )
                    nc.scalar.activation(
                        bt[:, hf * 512 : (hf + 1) * 512], pb, AF.Copy
                    )
                bc[nm] = bt

        finalize_mod(0, F, 0, 1, "Wa_bc", "Ba_bc")

        # ---------------- phase 1: adaLN-1 + attn-LN + kv + gather ----------------

        hT = [
            persist.tile([128, R], bf16, tag=f"hT{ft}", name=f"hT{ft}")
            for ft in range(FT)
        ]
        h_res = [hera.tile([128, F], bf16, name=f"h{rb}") for rb in range(RB)]
        kvT_sb = hera.tile([128, R], bf16, name="kvT_sb")

        kv_bounce = [dramp.tile([2 * D, 256], bf16, name=f"kvb{c}") for c in range(2)]
        kv_all = [dramp.tile([4 * 2 * D, 256], bf16, name=f"kva{c}") for c in range(2)]

        def layer_norm_stats(src):
            stats = work.tile([128, 2, 6], f32, tag="stats", name="stats")
            for sg in range(2):
                nc.vector.bn_stats(
                    out=stats[:, sg, :], in_=src[:, sg * 512 : (sg + 1) * 512]
                )
            mv = work.tile([128, 2], f32, tag="mv", name="mv")
            nc.vector.bn_aggr(out=mv, in_=stats)
            rstd = work.tile([128, 1], f32, tag="rstd", name="rstd")
            nc.scalar.activation(
                out=rstd, in_=mv[:, 1:2], func=AF.Sqrt, bias=epst, scale=1.0
            )
            nc.vector.reciprocal(out=rstd, in_=rstd)
            return mv, rstd

        def transpose_to_h2(hsrc_bf, rb, tpool):
            for ft in range(FT):
                pt = tpool.tile([128, 128], bf16, tag="pt", bufs=2, name="ptt2")
                nc.tensor.transpose(
                    pt, hsrc_bf[:, ft * 128 : (ft + 1) * 128], ident
                )
                nc.scalar.activation(
                    out=h2all[:, ft, rb * 128 : (rb + 1) * 128],
                    in_=pt,
                    func=AF.Copy,
                )

        def transpose_to(hsrc_bf, hT_tiles, rb, tpool):
            for ft in range(FT):
                pt = tpool.tile([128, 128], bf16, tag="pt", bufs=2, name="ptt")
                nc.tensor.transpose(
                    pt, hsrc_bf[:, ft * 128 : (ft + 1) * 128], ident
                )
                nc.scalar.activation(
                    out=hT_tiles[ft][:, rb * 128 : (rb + 1) * 128],
                    in_=pt,
                    func=AF.Copy,
                )

        with nc.named_scope("p1_ln"):
            xns = []
            for rb in range(RB):
                x_rb = work.tile([128, F], f32, tag="x", bufs=3, name="x_rb")
                nc.sync.dma_start(out=x_rb, in_=x_d[rb * 128 : (rb + 1) * 128, :])
                mv, rstd = layer_norm_stats(x_rb)
                xn = hera.tile([128, F], bf16, name=f"xnp{rb}")
                nc.vector.tensor_scalar(
                    out=xn,
                    in0=x_rb,
                    scalar1=mv[:, 0:1],
                    scalar2=rstd,
                    op0=OP.subtract,
                    op1=OP.mult,
                )
                xns.append(xn)
            for rb in range(RB):
                h0 = work.tile([128, F], bf16, tag="h0", name="h0")
                nc.vector.tensor_mul(out=h0, in0=xns[rb], in1=bc["Wa_bc"])
                nc.vector.tensor_add(out=h_res[rb], in0=h0, in1=bc["Ba_bc"])
                # attn-LN
                mv2, rstd2 = layer_norm_stats(h_res[rb])
                xn2 = work.tile([128, F], bf16, tag="xn", name="xn2")
                nc.vector.tensor_scalar(
                    out=xn2,
                    in0=h_res[rb],
                    scalar1=mv2[:, 0:1],
                    scalar2=rstd2,
                    op0=OP.subtract,
                    op1=OP.mult,
                )
                hn1 = work.tile([128, F], bf16, tag="h0", name="hn1")
                nc.vector.tensor_mul(out=hn1, in0=xn2, in1=anw_bc)
                hn_bf = work.tile([128, F], bf16, tag="hnbf", name="hn_bf")
                nc.vector.tensor_add(out=hn_bf, in0=hn1, in1=anb_bc)
                transpose_to(hn_bf, hT, rb, psF)
                # kv projection for this row block
                pkv = psF.tile([128, 512], f32, tag="pkq", bufs=2, name="pkv")
                for kt in range(FT):
                    nc.tensor.matmul(
                        pkv[:, 0:128],
                        wkv_sb[:, kt, :],
                        hT[kt][:, rb * 128 : (rb + 1) * 128],
                        start=(kt == 0),
                        stop=(kt == FT - 1),
                    )
                nc.scalar.activation(
                    out=kvT_sb[:, rb * 128 : (rb + 1) * 128],
                    in_=pkv[:, 0:128],
                    func=AF.Copy,
                )
                if rb % 2 == 1:
                    c = rb // 2
                    with nc.named_scope(f"gather{c}"):
                        nc.sync.dma_start(
                            out=kv_bounce[c], in_=kvT_sb[:, c * 256 : (c + 1) * 256]
                        )
                        nc.gpsimd.collective_compute(
                            "AllGather",
                            OP.bypass,
                            replica_groups=groups,
                            ins=[kv_bounce[c][:, :]],
                            outs=[kv_all[c][:, :]],
                        )

        finalize_mod(2 * F, 3 * F, 4, 5, "Wf_bc", "Bf_bc")
        cm_modtmp.__exit__(None, None, None)
        cm_aera = tc.tile_pool(name="aera", bufs=1)
        aera = cm_aera.__enter__()

        # ---------------- phase 2: q proj ----------------
        qT = [aera.tile([128, R], bf16, name=f"qT{mt}") for mt in range(MT)]
        with nc.named_scope("qproj"):
            for mt in range(MT):
                wqblk = wstr.tile(
                    [128, FT * 128], bf16, tag="wqb", bufs=2, name="wqblk"
                )
                nc.sync.dma_start(out=wqblk, in_=wq_d[mt])
                pq = psF.tile([128, 512], f32, tag="pkq", bufs=2, name="pq")
                for kt in range(FT):
                    nc.tensor.matmul(
                        pq,
                        wqblk[:, kt * 128 : (kt + 1) * 128],
                        hT[kt],
                        start=(kt == 0),
                        stop=(kt == FT - 1),
                    )
                # fold the attention 1/sqrt(D)=0.125 scale into q
                nc.scalar.activation(out=qT[mt], in_=pq, func=AF.Copy, scale=0.125)

        # ---------------- phase 3: kT / v_ext assembly ----------------
        # k^T duplicated into both partition halves so MM1's lhsT can share
        # the rhs (q head slice) base partition for even and odd heads.
        kT = aera.tile([128, T], bf16, name="kT")
        # fp8 DoubleRow PV weights: vd[j] holds 16*v for key tiles (2j, 2j+1)
        # in [Ki, ko, col] layout, col 64 = 16.0 (the softmax-denominator ones
        # column, pre-scaled to keep v in fp8 normal range).
        vd = [aera.tile([128, 2, 80], fp8, name=f"vd{j}") for j in range(KT // 2)]
        with nc.named_scope("asm"):
            for j in range(KT // 2):
                nc.vector.memset(vd[j][:, :, 64:65], 16.0)
            for c in range(2):
                for r in range(4):
                    for hp in (0, 64):
                        nc.sync.dma_start(
                            out=kT[
                                hp : hp + 64, r * 512 + c * 256 : r * 512 + (c + 1) * 256
                            ],
                            in_=kv_all[c][r * 128 : r * 128 + 64, :],
                        )
                    vT_sb = work.tile([64, 256], bf16, tag="vTs", name="vT_sb")
                    nc.sync.dma_start(
                        out=vT_sb, in_=kv_all[c][r * 128 + 64 : (r + 1) * 128, :]
                    )
                    for cc in range(2):
                        kt = r * 4 + c * 2 + cc
                        ptv = psF.tile(
                            [128, 128], bf16, tag="pt", bufs=2, name="ptv"
                        )
                        nc.tensor.matmul(
                            ptv[:, 0:64],
                            vT_sb[:, cc * 128 : (cc + 1) * 128],
                            ident[0:64, 0:64],
                            is_transpose=True,
                        )
                        nc.scalar.activation(
                            out=vd[kt // 2][:, kt % 2, 0:64],
                            in_=ptv[:, 0:64],
                            func=AF.Copy,
                            scale=16.0,
                        )

        # prefetch wo during attention (sync queue)
        if V_OPACK == "dma":
            woc = [aera.tile([128, F], bf16, name=f"woc{m}") for m in range(MT)]
            for m in range(MT):
                nc.sync.dma_start(out=woc[m], in_=wo_d[m * 128 : (m + 1) * 128, :])
        else:
            woc_f = [aera.tile([64, F], bf16, name=f"wocf{k}") for k in range(H)]
            for k in range(H):
                nc.sync.dma_start(
                    out=woc_f[k], in_=wo_d[k * 64 : (k + 1) * 64, :]
                )

        cm_psF.__exit__(None, None, None)
        cm_psAT = tc.tile_pool(name="psAT", bufs=1, space="PSUM")
        psAT = cm_psAT.__enter__()

        # ---------------- phase 4: attention ----------------
        # Per head: MM1 into [128,1024] psum tiles (2 key tiles per bank
        # pair), exp at FD=1024 psum->sbuf bf16, PV with ones-column denom.
        # PE instruction order interleaves MM1(h) with PV(h-1) so the queue
        # head is never blocked on exp.
        if V_OPACK == "dma":
            outTp = [aera.tile([128, R], bf16, name=f"outTp{m}") for m in range(MT)]
        else:
            outT_f = [aera.tile([64, R], bf16, name=f"outTf{k}") for k in range(H)]
        probs_all = {}
        probs_sc = {}

        def emit_mm1(hi, j):
            # scores for key tiles (2j, 2j+1) of head hi; two 1-bank psum
            # tiles exp'd separately into halves of the fp8 probs pair tile
            mt, hp = hi // 2, (hi % 2) * 64
            pr = aera.tile(
                [128, 2, 512], fp8, tag="probs", bufs=20, name=f"pr{hi}_{j}"
            )
            for cc in range(2):
                kt = 2 * j + cc
                sc = psAT.tile(
                    [128, 512], f32, tag="sc", bufs=4, name=f"sc{hi}_{j}_{cc}"
                )
                nc.tensor.matmul(
                    sc,
                    kT[hp : hp + 64, kt * 128 : (kt + 1) * 128],
                    qT[mt][hp : hp + 64, :],
                    start=True,
                    stop=True,
                )
                if hi % 2 == 1 and cc == 1:
                    # Schraudolph fast-exp on DVE: exp(s) ~ bits(A*s + B) as
                    # fp32 (~3% sawtooth, same order as the fp8 probs quant;
                    # softmax averaging washes it out). Offloads ~25% of the
                    # exp work from the saturated ScalarE.
                    ti = aera.tile(
                        [128, 512], mybir.dt.int32, tag="ti", bufs=2, name="ti"
                    )
                    with nc.allow_low_precision(reason="approx exp for probs"):
                        nc.vector.tensor_scalar(
                            out=ti,
                            in0=sc,
                            scalar1=12102203.161561485,
                            scalar2=1064866805.0,
                            op0=OP.mult,
                            op1=OP.add,
                        )
                        nc.vector.tensor_copy(
                            out=pr[:, cc, :], in_=ti[:, :].bitcast(f32)
                        )
                else:
                    nc.scalar.activation(out=pr[:, cc, :], in_=sc, func=AF.Exp)
            probs_all[(hi, j)] = pr

        def emit_pv(hi, j, po):
            # fp8 DoubleRow: contracts key tiles 2j and 2j+1 in one matmul
            pr = probs_all.pop((hi, j))
            nc.tensor.matmul(
                po[0:65, :],
                vd[j][:, :, 0:65],
                pr,
                start=(j == 0),
                stop=(j == 7),
                perf_mode=DR,
            )

        def emit_normalize(hi, po):
            # reciprocal of the denominator row, broadcast to 64 partitions
            # via a DRAM bounce (no PE involvement -> no PE queue blocking)
            m, hp = hi // 2, (hi % 2) * 64
            # 1/denom = exp(-ln(denom)) on ScalarE (exp+ln share one ACT
            # table set; DVE's iterative reciprocal would block the probs
            # pipeline for ~3.3us per head)
            lnu = aera.tile([128, R], f32, tag="lnu", bufs=2, name="lnu")
            nc.scalar.activation(out=lnu[64:65, :], in_=po[64:65, :], func=AF.Ln)
            rcp_row = aera.tile([128, R], f16, tag="rcp", bufs=2, name="rcp_row")
            nc.scalar.activation(
                out=rcp_row[64:65, :], in_=lnu[64:65, :], func=AF.Exp, scale=-1.0
            )
            rden = dramp.tile([1, R], f16, tag="rden", bufs=2, name=f"rden{hi}")
            nc.sync.dma_start(out=rden, in_=rcp_row[64:65, :])
            rcpb = aera.tile([64, R], f16, tag="rcpb", bufs=2, name="rcpb")
            nc.sync.dma_start(out=rcpb, in_=bcast_row(rden[0:1, :], nparts=64))
            if hp == 0:
                nc.vector.tensor_mul(
                    out=outTp[m][0:64, :], in0=po[0:64, :], in1=rcpb
                )
            else:
                # DVE cannot shift partitions; stage at base 0 then DMA up
                oT = aera.tile([64, R], bf16, tag="oT", bufs=2, name="oT")
                nc.vector.tensor_mul(out=oT, in0=po[0:64, :], in1=rcpb)
                nc.sync.dma_start(out=outTp[m][64:128, :], in_=oT)

        JSEQ = [0, 2, 4, 6, 1, 3, 5, 7]  # chunk-0 key tiles first
        with nc.named_scope("attn"):
            po_t = {}
            for j in JSEQ:
                emit_mm1(0, j)
                emit_mm1(1, j)
            for m in range(MT):
                h0, h1 = 2 * m, 2 * m + 1
                if m > 0:
                    emit_normalize(h0 - 2, po_t.pop(h0 - 2))
                    emit_normalize(h1 - 2, po_t.pop(h1 - 2))
                for hi in (h0, h1):
                    po_t[hi] = psAT.tile(
                        [128, 512], f32, tag="po", bufs=4, name=f"po{hi}"
                    )
                for j in JSEQ:
                    emit_pv(h0, j, po_t[h0])
                    emit_pv(h1, j, po_t[h1])
                    if m + 1 < MT:
                        emit_mm1(h0 + 2, j)
                        emit_mm1(h1 + 2, j)
            emit_normalize(H - 2, po_t.pop(H - 2))
            emit_normalize(H - 1, po_t.pop(H - 1))

        cm_psAT.__exit__(None, None, None)
        cm_psO = tc.tile_pool(name="psO", bufs=1, space="PSUM")
        psO = cm_psO.__enter__()

        # ---------------- phase 5: out proj (head pairs) + residual -> x1 ----------------
        x1 = [persist.tile([128, F], f32, name=f"x1_{rt}") for rt in range(RB)]
        with nc.named_scope("oproj"):
            px1 = {}
            for rt in range(RB):
                px1[rt] = psO.tile([128, F], f32, tag="px1", bufs=4, name=f"px1_{rt}")
            nchunk = MT if V_OPACK == "dma" else H
            for m in range(nchunk):
                for rt in range(RB):
                    for nh in range(2):
                        if V_OPACK == "dma":
                            lhsT = outTp[m][:, rt * 128 : (rt + 1) * 128]
                            rhs = woc[m][:, nh * 512 : (nh + 1) * 512]
                        else:
                            lhsT = outT_f[m][:, rt * 128 : (rt + 1) * 128]
                            rhs = woc_f[m][:, nh * 512 : (nh + 1) * 512]
                        nc.tensor.matmul(
                            px1[rt][:, nh * 512 : (nh + 1) * 512],
                            lhsT,
                            rhs,
                            start=(m == 0),
                            stop=(V_BIAS == "dve" and m == nchunk - 1),
                        )
            for rt in range(RB):
                if V_BIAS == "mm":
                    for nh in range(2):
                        # fold wo_b into the accumulator: += ones^T(rows) x wob
                        nc.tensor.matmul(
                            px1[rt][:, nh * 512 : (nh + 1) * 512],
                            ones_b,
                            wob_row[:, nh * 512 : (nh + 1) * 512],
                            start=False,
                            stop=True,
                        )
                nc.vector.tensor_add(out=x1[rt], in0=px1[rt], in1=h_res[rt])
                if V_BIAS == "dve":
                    nc.vector.tensor_add(out=x1[rt], in0=x1[rt], in1=wob_bc)

        cm_aera.__exit__(None, None, None)
        cm_hera.__exit__(None, None, None)

        cm_psO.__exit__(None, None, None)
        cm_psM = tc.tile_pool(name="psM", bufs=1, space="PSUM")
        psM = cm_psM.__enter__()

        # ---------------- phase 6: adaLN-2 + transpose ----------------
        h2all = persist.tile([128, FT, R], fp8 if V_MLP8 else bf16, name="h2all")
        with nc.named_scope("aln2"):
            for rt in range(RB):
                mv, rstd = layer_norm_stats(x1[rt])
                xn = work.tile([128, F], bf16, tag="xn", name="xn3")
                nc.vector.tensor_scalar(
                    out=xn,
                    in0=x1[rt],
                    scalar1=mv[:, 0:1],
                    scalar2=rstd,
                    op0=OP.subtract,
                    op1=OP.mult,
                )
                h21 = work.tile([128, F], bf16, tag="h0", name="h21")
                nc.vector.tensor_mul(out=h21, in0=xn, in1=bc["Wf_bc"])
                h2_bf = work.tile([128, F], bf16, tag="hnbf", name="h2_bf")
                nc.vector.tensor_add(out=h2_bf, in0=h21, in1=bc["Bf_bc"])
                transpose_to_h2(h2_bf, rt, psM)

        # ---------------- phase 7+8: mlp1 + gelu, mlp2 interleaved ----------------
        cm_mlp = tc.tile_pool(name="mlpera", bufs=1)
        mlpera = cm_mlp.__enter__()
        g1all = mlpera.tile([128, MFT, R], fp8 if V_MLP8 else bf16, name="g1all")
        pf = {}
        with nc.named_scope("mlp"):
            if V_MLP8:
                w2f = [
                    mlpera.tile([128, 2, F], fp8, name=f"w2f{k}")
                    for k in range(MFT // 2)
                ]
                for k in range(MFT // 2):
                    eng = nc.gpsimd if k % 2 == 0 else nc.scalar
                    eng.dma_start(out=w2f[k], in_=w2_d[k])
            for rt in range(RB):
                pf[rt] = psM.tile([128, 512], f32, tag="pf", bufs=4, name=f"pf{rt}")

            def mlp2_chunk8(k, fh, pfd):
                # fp8 DR mlp2 accumulation for hidden pair k, F-half fh
                for rt in range(RB):
                    nc.tensor.matmul(
                        pfd[rt],
                        g1all[:, 2 * k : 2 * k + 2, rt * 128 : (rt + 1) * 128],
                        w2f[k][:, :, fh * 512 : (fh + 1) * 512],
                        start=(k == 0),
                        stop=False,
                        perf_mode=DR,
                    )

            def mlp2_chunk16(mt, fh, pfd):
                # bf16 mlp2 accumulation for hidden tile mt
                w2c = wstr.tile([128, 512], bf16, tag="w2c", bufs=3, name="w2c")
                eng = nc.gpsimd if mt % 2 == 0 else nc.scalar
                eng.dma_start(
                    out=w2c,
                    in_=w2_d[mt * 128 : (mt + 1) * 128, fh * 512 : (fh + 1) * 512],
                )
                for rt in range(RB):
                    nc.tensor.matmul(
                        pfd[rt],
                        g1all[:, mt, rt * 128 : (rt + 1) * 128],
                        w2c,
                        start=(mt == 0),
                        stop=False,
                    )

            for mt in range(MFT):
                if V_MLP8:
                    w1blk = wstr.tile(
                        [128, FT // 2, 2, 128], fp8, tag="w1b", bufs=3, name="w1blk"
                    )
                    eng = nc.sync if mt % 2 == 0 else nc.scalar
                    eng.dma_start(
                        out=w1blk,
                        in_=w1_d[mt].rearrange(
                            "p (j two m) -> p j two m", j=FT // 2, two=2
                        ),
                    )
                else:
                    w1blk = wstr.tile(
                        [128, FT * 128], bf16, tag="w1b", bufs=3, name="w1blk"
                    )
                    eng = nc.sync if mt % 2 == 0 else nc.scalar
                    eng.dma_start(out=w1blk, in_=w1_d[mt])
                pg = psM.tile([128, 512], f32, tag="pg", bufs=2, name="pg")
                if V_MLP8:
                    for j in range(FT // 2):
                        nc.tensor.matmul(
                            pg,
                            w1blk[:, j],
                            h2all[:, 2 * j : 2 * j + 2, :],
                            start=(j == 0),
                            stop=(j == FT // 2 - 1),
                            perf_mode=DR,
                        )
                else:
                    for kt in range(FT):
                        nc.tensor.matmul(
                            pg,
                            w1blk[:, kt * 128 : (kt + 1) * 128],
                            h2all[:, kt, :],
                            start=(kt == 0),
                            stop=(kt == FT - 1),
                        )
                nc.scalar.activation(
                    out=g1all[:, mt, :],
                    in_=pg,
                    func=AF.Gelu,
                    bias=b1_sb[:, mt : mt + 1],
                    scale=(1.0 / 256.0) if V_MLP8 else 1.0,
                )
                if V_MLP8:
                    if mt % 2 == 1 and mt > 1:
                        mlp2_chunk8((mt - 2) // 2, 0, pf)
                elif mt > 0:
                    mlp2_chunk16(mt - 1, 0, pf)
            if V_MLP8:
                mlp2_chunk8(MFT // 2 - 1, 0, pf)
            else:
                mlp2_chunk16(MFT - 1, 0, pf)
            oscale = (1.0 / 256.0) if V_MLP8 else 1.0
            for rt in range(RB):
                nc.tensor.matmul(
                    pf[rt], ones_b, b2_row[:, 0:512], start=False, stop=True
                )
                yh1 = work.tile([128, 512], f32, tag="yh1", name="yh1")
                nc.scalar.activation(out=yh1, in_=pf[rt], func=AF.Copy, scale=oscale)
                yh = work.tile([128, 512], f32, tag="yh", name="yh")
                nc.vector.tensor_add(out=yh, in0=yh1, in1=x1[rt][:, 0:512])
                nc.sync.dma_start(out=y_d[rt * 128 : (rt + 1) * 128, 0:512], in_=yh)
            # second half of mlp2
            pf2 = {}
            for rt in range(RB):
                pf2[rt] = psM.tile([128, 512], f32, tag="pf", bufs=4, name=f"pf2{rt}")
            if V_MLP8:
                for k in range(MFT // 2):
                    mlp2_chunk8(k, 1, pf2)
            else:
                for mt in range(MFT):
                    mlp2_chunk16(mt, 1, pf2)
            for rt in range(RB):
                nc.tensor.matmul(
                    pf2[rt], ones_b, b2_row[:, 512:1024], start=False, stop=True
                )
                yh1 = work.tile([128, 512], f32, tag="yh1", name="yh1b")
                nc.scalar.activation(out=yh1, in_=pf2[rt], func=AF.Copy, scale=oscale)
                yh = work.tile([128, 512], f32, tag="yh", name="yh2")
                nc.vector.tensor_add(out=yh, in0=yh1, in1=x1[rt][:, 512:1024])
                nc.sync.dma_start(
                    out=y_d[rt * 128 : (rt + 1) * 128, 512:1024], in_=yh
                )
        cm_mlp.__exit__(None, None, None)
        cm_psM.__exit__(None, None, None)

    nc.compile()
    return nc


def _prep_in_maps(inputs):
    f32 = np.float32
    wmod = np.concatenate(
        [inputs["amod_gw"], inputs["amod_bw"], inputs["fmod_gw"], inputs["fmod_bw"]],
        axis=1,
    ).astype(BF16)
    modb = np.concatenate(
        [inputs["amod_gb"], inputs["amod_bb"], inputs["fmod_gb"], inputs["fmod_bb"]]
    ).astype(f32)
    lnvec = np.stack(
        [
            inputs["amod_nw"],
            inputs["amod_nb"],
            inputs["attn_nw"],
            inputs["attn_nb"],
            inputs["fmod_nw"],
            inputs["fmod_nb"],
        ]
    ).astype(f32)
    wq_t = np.ascontiguousarray(
        np.asarray(inputs["wq"]).astype(BF16).reshape(FT, 128, MT, 128)
        .transpose(2, 1, 0, 3).reshape(MT, 128, FT * 128)
    )
    FP8 = ml_dtypes.float8_e4m3
    if int(os.environ.get("V_MLP8", "1")):
        # w1 * 256 in DoubleRow layout [mt, i, (j two m)]
        w1_t = np.ascontiguousarray(
            np.clip(np.asarray(inputs["w1"], np.float32) * 256.0, -240, 240)
            .reshape(FT // 2, 2, 128, MFT, 128)
            .transpose(3, 2, 0, 1, 4).reshape(MFT, 128, FT * 128)
        ).astype(FP8)
        # w2 * 256 in DoubleRow rhs layout [k, i, (two f)]
        w2_t = np.ascontiguousarray(
            np.clip(np.asarray(inputs["w2"], np.float32) * 256.0, -240, 240)
            .reshape(MFT // 2, 2, 128, F)
            .transpose(0, 2, 1, 3).reshape(MFT // 2, 128, 2 * F)
        ).astype(FP8)
        b2_t = np.asarray(inputs["b2"]).astype(f32).reshape(1, F) * 256.0
    else:
        w1_t = np.ascontiguousarray(
            np.asarray(inputs["w1"]).astype(BF16).reshape(FT, 128, MFT, 128)
            .transpose(2, 1, 0, 3).reshape(MFT, 128, FT * 128)
        )
        w2_t = np.asarray(inputs["w2"]).astype(BF16)
        b2_t = np.asarray(inputs["b2"]).astype(f32).reshape(1, F)
    shared = dict(
        wmod=wmod,
        modb=modb,
        lnvec=lnvec,
        wq=wq_t,
        wkv=np.asarray(inputs["wkv"]).astype(BF16),
        wo=np.asarray(inputs["wo"]).astype(BF16),
        wo_bias=np.asarray(inputs["wo_b"]).astype(f32).reshape(1, F),
        w1=w1_t,
        b1=np.asarray(inputs["b1"]).astype(f32),
        w2=w2_t,
        b2=b2_t,
    )
    x = np.asarray(inputs["x"]).astype(f32)
    cond = np.asarray(inputs["cond"]).astype(BF16)
    in_maps = []
    for c in range(NCORES):
        b, r0 = c // 4, (c % 4) * R
        m = dict(shared)
        m["x"] = np.ascontiguousarray(x[b, r0 : r0 + R, :])
        m["cond"] = np.ascontiguousarray(cond[b])
        in_maps.append(m)
    return in_maps


def _run(inputs, trace=False, tmpdir=None):
    from concourse.bass_utils import run_bass_kernel_spmd

    if "nc" not in _CACHE:
        _CACHE["nc"] = _build_nc()
    nc = _CACHE["nc"]
    in_maps = _prep_in_maps(inputs)
    res = run_bass_kernel_spmd(
        nc, in_maps, core_ids=list(range(NCORES)), trace=trace, tmpdir=tmpdir
    )
    y = np.empty((B, T, F), np.float32)
    for c in range(NCORES):
        b, r0 = c // 4, (c % 4) * R
        y[b, r0 : r0 + R, :] = res.results[c]["y"]
    return y, res


def kernel(**inputs) -> np.ndarray:
    y, _ = _run(inputs, trace=False)
    return y


if __name__ == "__main__":
    _build_nc()
    print("build OK")
